# revision 18
# baseline (speedup 1.0000x reference)
"""Trainium2 Bass kernel for a dense transformer block (pre-LN attention + GELU MLP).

Strategy: data-parallel over batch across 8 NeuronCores (2 batches/core, no
collectives).  Per core: token-major residual stream with feature-major
activations for matmuls (PE-transpose at the two LayerNorms), fp32r matmuls
(full PE rate), softmax without max-subtraction (scores are O(1) bounded by
construction), PV matmul with a ones-column on V to produce row-sums for free.
"""

import numpy as np

import concourse.bass as bass
import concourse.mybir as mybir
import concourse.tile as tile
from concourse import bacc, bass_utils
from concourse.masks import make_identity

# Problem shape (hardcoded per spec nn_Block_58652073394865)
B, S, D, H, F = 16, 577, 1024, 16, 4096
DH = D // H
NCORES = 8
BL = B // NCORES        # batches per core
P = 128
KK = D // P             # 8 chunks of the model dim
FK = F // P             # 32 chunks of the mlp dim
EPS = 1e-6

# fp32r matmuls require even free-dim counts, so pad tokens 577 -> 578 (one
# zeroed pad token) and use even, overlapping moving-token chunks.
SP = 578
TT = [(0, 128), (128, 128), (256, 128), (384, 128), (512, 66)]   # token tiles (incl pad)
QC = [(0, 290), (288, 290)]                                      # moving-token chunks (even, >=256)
DC = [(0, 512), (512, 512)]                                      # model-dim 512 chunks
VS = 66                                                          # per-head stride in v (64 v + 1 ones + 1 pad)

F32 = mybir.dt.float32
F32R = mybir.dt.float32r
AF = mybir.ActivationFunctionType
OP = mybir.AluOpType

WEIGHT_NAMES = [
    "ln1_g", "ln1_b", "wq", "bq", "wk", "bk", "wv", "bv", "wo", "bo",
    "ln2_g", "ln2_b", "w1", "b1", "w2", "b2",
]

_NC_CACHE = None
# CoreSim doesn't implement the Gelu LUT; tests may swap this for AF.Tanh
_GELU = AF.Gelu


def _build():
    nc = bacc.Bacc("TRN2", target_bir_lowering=False, debug=False,
                   num_devices=NCORES)

    x_d = nc.dram_tensor("x", [BL, S, D], F32, kind="ExternalInput").ap()
    y_d = nc.dram_tensor("y", [BL, S, D], F32, kind="ExternalOutput").ap()
    # weights consumed by matmuls -> declare fp32r (same bits as fp32)
    wq_d = nc.dram_tensor("wq", [D, D], F32R, kind="ExternalInput").ap()
    wk_d = nc.dram_tensor("wk", [D, D], F32R, kind="ExternalInput").ap()
    wv_d = nc.dram_tensor("wv", [D, D], F32R, kind="ExternalInput").ap()
    wo_d = nc.dram_tensor("wo", [D, D], F32R, kind="ExternalInput").ap()
    w1_d = nc.dram_tensor("w1", [D, F], F32R, kind="ExternalInput").ap()
    w2_d = nc.dram_tensor("w2", [F, D], F32R, kind="ExternalInput").ap()
    bv_d = nc.dram_tensor("bv", [D], F32R, kind="ExternalInput").ap()   # folded via K=1 matmul
    bo_d = nc.dram_tensor("bo", [D], F32R, kind="ExternalInput").ap()   # folded via K=1 matmul
    bq_d = nc.dram_tensor("bq", [D], F32, kind="ExternalInput").ap()
    bk_d = nc.dram_tensor("bk", [D], F32, kind="ExternalInput").ap()
    b1_d = nc.dram_tensor("b1", [F], F32, kind="ExternalInput").ap()
    b2_d = nc.dram_tensor("b2", [D], F32, kind="ExternalInput").ap()
    g1_d = nc.dram_tensor("ln1_g", [D], F32, kind="ExternalInput").ap()
    gb1_d = nc.dram_tensor("ln1_b", [D], F32, kind="ExternalInput").ap()
    g2_d = nc.dram_tensor("ln2_g", [D], F32, kind="ExternalInput").ap()
    gb2_d = nc.dram_tensor("ln2_b", [D], F32, kind="ExternalInput").ap()

    wq_r = wq_d.rearrange("(ko p) d -> p ko d", p=P)
    wk_r = wk_d.rearrange("(ko p) d -> p ko d", p=P)
    wv_r = wv_d.rearrange("(ko p) d -> p ko d", p=P)
    wo_r = wo_d.rearrange("(ko p) d -> p ko d", p=P)
    w1_r = w1_d.rearrange("(ko p) d -> p ko d", p=P)
    w2_r = w2_d.rearrange("(ko p) d -> p ko d", p=P)

    with tile.TileContext(nc) as tc:
        with tc.tile_pool(name="const", bufs=1) as cpool, \
             tc.tile_pool(name="resid", bufs=2) as rpool, \
             tc.tile_pool(name="fmbuf", bufs=1) as fmpool, \
             tc.tile_pool(name="ostg", bufs=4) as opool, \
             tc.tile_pool(name="psA", bufs=8, space="PSUM") as psA:

            # ---- constants / small params ----
            ident = cpool.tile([P, P], F32, tag="ident")
            make_identity(nc, ident[:])
            epsap = cpool.tile([P, 1], F32, tag="eps")
            nc.vector.memset(epsap[:], EPS)
            ones_f = cpool.tile([1, P], F32, tag="ones_f")
            nc.vector.memset(ones_f[:], 1.0)
            ones_r = cpool.tile([1, P], F32R, tag="ones_r")
            nc.vector.tensor_copy(ones_r[:], ones_f[:])
            onec_f = cpool.tile([P, 1], F32, tag="onec_f")
            nc.vector.memset(onec_f[:], 1.0)

            bq_sb = cpool.tile([P, KK], F32, tag="bq")
            nc.sync.dma_start(bq_sb[:], bq_d.rearrange("(m p) -> p m", p=P))
            bk_sb = cpool.tile([P, KK], F32, tag="bk")
            nc.sync.dma_start(bk_sb[:], bk_d.rearrange("(m p) -> p m", p=P))
            b1_sb = cpool.tile([P, FK], F32, tag="b1")
            nc.sync.dma_start(b1_sb[:], b1_d.rearrange("(m p) -> p m", p=P))
            b2_sb = cpool.tile([P, KK], F32, tag="b2")
            nc.sync.dma_start(b2_sb[:], b2_d.rearrange("(m p) -> p m", p=P))
            g1_sb = cpool.tile([P, KK], F32, tag="g1")
            nc.sync.dma_start(g1_sb[:], g1_d.rearrange("(c p) -> p c", p=P))
            gb1_sb = cpool.tile([P, KK], F32, tag="gb1")
            nc.sync.dma_start(gb1_sb[:], gb1_d.rearrange("(c p) -> p c", p=P))
            g2_sb = cpool.tile([P, KK], F32, tag="g2")
            nc.sync.dma_start(g2_sb[:], g2_d.rearrange("(c p) -> p c", p=P))
            gb2_sb = cpool.tile([P, KK], F32, tag="gb2")
            nc.sync.dma_start(gb2_sb[:], gb2_d.rearrange("(c p) -> p c", p=P))
            t_bo = cpool.tile([1, D], F32R, tag="t_bo")
            nc.sync.dma_start(t_bo[:], bo_d[None, :])
            t_bv = cpool.tile([1, D], F32R, tag="t_bv")
            nc.sync.dma_start(t_bv[:], bv_d[None, :])

            # token-major layernorm -> feature-major normalized output
            def layer_norm_fm(ln_pool, src, g_sb, gb_sb, dst_fm):
                negmu = ln_pool.tile([P, 5], F32, tag="negmu")
                varD = ln_pool.tile([P, 5], F32, tag="varD")
                sig = ln_pool.tile([P, 5], F32, tag="sig")
                rsig = ln_pool.tile([P, 5], F32, tag="rsig")
                # last token tile covers only 65 partitions; keep the rest defined
                nc.vector.memset(negmu[:], 0.0)
                nc.vector.memset(varD[:], 1.0)
                for ti, (t0, pt) in enumerate(TT):
                    nc.vector.tensor_reduce(
                        negmu[:pt, ti:ti + 1], src[:pt, ti],
                        mybir.AxisListType.X, OP.add)
                # pad token row of the last tile is zero -> stats stay finite
                nc.vector.tensor_scalar_mul(negmu[:], negmu[:], -1.0 / D)
                for ti, (t0, pt) in enumerate(TT):
                    scr = ln_pool.tile([P, D], F32, tag="xn_tm")
                    nc.scalar.activation(
                        scr[:pt], src[:pt, ti], AF.Square,
                        bias=negmu[:pt, ti:ti + 1], accum_out=varD[:pt, ti:ti + 1])
                nc.scalar.activation(sig[:], varD[:], AF.Sqrt,
                                     scale=1.0 / D, bias=epsap[:])
                nc.vector.reciprocal(rsig[:], sig[:])
                for ti, (t0, pt) in enumerate(TT):
                    xn = ln_pool.tile([P, D], F32, tag="xn_tm")
                    nc.vector.tensor_scalar(
                        xn[:pt], src[:pt, ti],
                        negmu[:pt, ti:ti + 1], rsig[:pt, ti:ti + 1],
                        OP.add, OP.mult)
                    for kk in range(KK):
                        pst = psA.tile([P, 512], F32, tag="pA")
                        nc.tensor.transpose(
                            pst[:, :pt], xn[:pt, kk * P:(kk + 1) * P],
                            ident[:pt, :pt])
                        nc.vector.scalar_tensor_tensor(
                            dst_fm[:, kk, t0:t0 + pt], pst[:, :pt],
                            g_sb[:, kk:kk + 1],
                            gb_sb[:, kk:kk + 1].to_broadcast((P, pt)),
                            OP.mult, OP.add)

            for b in range(BL):
                xn_fm = fmpool.tile([P, KK, SP], F32R, tag="xn_fm")
                xb = rpool.tile([P, 5, D], F32, tag="resid")

                # ---- stage A: load x (token-major); zero the pad token row ----
                # (engine start-partition must be a multiple of 32: zero 64..127
                # first, then the DMA rewrites the real rows 0..64)
                nc.vector.memset(xb[64:, 4, :], 0.0)
                for ti, (t0, pt) in enumerate(TT):
                    rp = min(pt, S - t0)   # real (non-pad) tokens in this tile
                    nc.sync.dma_start(xb[:rp, ti], x_d[b, t0:t0 + rp, :])

                # ---- stage B: LN1 -> xn_fm ----
                with tc.tile_pool(name="ln1", bufs=2) as lnp:
                    layer_norm_fm(lnp, xb, g1_sb, gb1_sb, xn_fm)

                with tc.tile_pool(name="attn", bufs=1) as apool, \
                     tc.tile_pool(name="wblk", bufs=2) as wpool:
                    q_fm = apool.tile([P, KK, SP], F32R, tag="q")
                    k_fm = apool.tile([P, KK, SP], F32R, tag="k")
                    v_sb = apool.tile([P, 5, H * VS], F32R, tag="v")
                    ctx_fm = apool.tile([P, KK, SP], F32R, tag="ctx")

                    # col 64 of each head's stride-66 group = 1 (rowsum trick),
                    # col 65 = 0 (fp32r even-M pad).  The pad token's whole v
                    # row (tile 4, partition 65) must be zero: zero partitions
                    # 64.. first, later writes refill only the real rows.
                    v_hc = v_sb[:].rearrange("p t (h c) -> p t h c", c=VS)
                    # memset can't target fp32r; zero via a uint32 view
                    nc.vector.memset(v_hc[64:, 4:5].bitcast(mybir.dt.uint32), 0)
                    nc.vector.memset(v_hc[:, :, :, 65:66].bitcast(mybir.dt.uint32), 0)
                    nc.vector.tensor_copy(
                        v_hc[:, 0:4, :, 64:65],
                        onec_f[:, :, None, None].to_broadcast((P, 4, H, 1)))
                    nc.vector.tensor_copy(
                        v_hc[:65, 4:5, :, 64:65],
                        onec_f[:65, :, None, None].to_broadcast((65, 1, H, 1)))

                    # ---- stage C: projections ----
                    # q, k: feature-major outputs
                    for w_r, bias_sb, dst in ((wq_r, bq_sb, q_fm), (wk_r, bk_sb, k_fm)):
                        for blk in range(2):
                            wb = wpool.tile([P, KK, 512], F32R, tag="wblk")
                            nc.sync.dma_start(wb[:], w_r[:, :, blk * 512:(blk + 1) * 512])
                            for mi in range(4):
                                m = blk * 4 + mi
                                for (q0, qn) in QC:
                                    ps = psA.tile([P, 512], F32, tag="pA")
                                    for kk in range(KK):
                                        nc.tensor.matmul(
                                            ps[:, :qn],
                                            wb[:, kk, mi * P:(mi + 1) * P],
                                            xn_fm[:, kk, q0:q0 + qn],
                                            start=(kk == 0), stop=(kk == KK - 1))
                                    nc.scalar.activation(
                                        dst[:, m, q0:q0 + qn], ps[:, :qn],
                                        AF.Identity, bias=bias_sb[:, m:m + 1])
                    # v: token-major output with interleaved ones columns
                    for ci, (c0, cn) in enumerate(DC):
                        wb = wpool.tile([P, KK, 512], F32R, tag="wblk")
                        nc.sync.dma_start(wb[:], wv_r[:, :, c0:c0 + cn])
                        for ti, (t0, pt) in enumerate(TT):
                            ps = psA.tile([P, 512], F32, tag="pA")
                            for kk in range(KK):
                                nc.tensor.matmul(
                                    ps[:pt], xn_fm[:, kk, t0:t0 + pt],
                                    wb[:, kk, :], start=(kk == 0), stop=False)
                            nc.tensor.matmul(
                                ps[:pt], ones_r[:, :pt], t_bv[:, c0:c0 + cn],
                                start=False, stop=True)
                            rp = min(pt, S - t0)
                            nc.vector.tensor_copy(
                                v_sb[:rp, ti].rearrange("p (h c) -> p h c", c=VS)[:, ci * 8:(ci + 1) * 8, 0:64],
                                ps[:rp].rearrange("p (h c) -> p h c", c=64))

                    # ---- stage D: attention ----
                    for h in range(H):
                        hrow = (h % 2) * 64
                        kkh = h // 2
                        for qi, (q0, qn) in enumerate(QC):
                            es = apool.tile([P, 5, qn], F32R, tag=f"es{qi}")
                            for kt, (t0, ptk) in enumerate(TT):
                                ps = psA.tile([P, 512], F32, tag="pA")
                                nc.tensor.matmul(
                                    ps[:ptk, :qn],
                                    k_fm[hrow:hrow + 64, kkh, t0:t0 + ptk],
                                    q_fm[hrow:hrow + 64, kkh, q0:q0 + qn],
                                    start=True, stop=True)
                                nc.scalar.activation(
                                    es[:ptk, kt, :], ps[:ptk, :qn],
                                    AF.Exp, scale=1.0 / np.sqrt(DH))
                            pc = psA.tile([VS, 512], F32, tag="pA")
                            for kt, (t0, ptk) in enumerate(TT):
                                nc.tensor.matmul(
                                    pc[:, :qn],
                                    v_sb[:ptk, kt, h * VS:(h + 1) * VS],
                                    es[:ptk, kt, :],
                                    start=(kt == 0), stop=(kt == 4))
                            rc = apool.tile([1, 290], F32, tag="rc")
                            nc.vector.reciprocal(rc[:, :qn], pc[64:65, :qn])
                            rb = apool.tile([64, 290], F32, tag="rb")
                            nc.gpsimd.partition_broadcast(rb[:, :qn], rc[:, :qn])
                            nc.vector.tensor_tensor(
                                ctx_fm[hrow:hrow + 64, kkh, q0:q0 + qn],
                                pc[0:64, :qn], rb[:, :qn], OP.mult)

                    # ---- stage E: output projection + residual -> x2 ----
                    x2 = rpool.tile([P, 5, D], F32, tag="resid")
                    for ci, (c0, cn) in enumerate(DC):
                        wb = wpool.tile([P, KK, 512], F32R, tag="wblk")
                        nc.sync.dma_start(wb[:], wo_r[:, :, c0:c0 + cn])
                        for ti, (t0, pt) in enumerate(TT):
                            ps = psA.tile([P, 512], F32, tag="pA")
                            for kk in range(KK):
                                nc.tensor.matmul(
                                    ps[:pt], ctx_fm[:, kk, t0:t0 + pt],
                                    wb[:, kk, :], start=(kk == 0), stop=False)
                            nc.tensor.matmul(
                                ps[:pt], ones_r[:, :pt], t_bo[:, c0:c0 + cn],
                                start=False, stop=True)
                            nc.vector.scalar_tensor_tensor(
                                x2[:pt, ti, c0:c0 + cn], ps[:pt], 0.0,
                                xb[:pt, ti, c0:c0 + cn], OP.add, OP.add)

                # ---- stage F: LN2 -> xn_fm (rotates) ----
                xn2_fm = fmpool.tile([P, KK, SP], F32R, tag="xn_fm")
                with tc.tile_pool(name="ln2", bufs=2) as lnp:
                    layer_norm_fm(lnp, x2, g2_sb, gb2_sb, xn2_fm)

                # ---- stage G: MLP ----
                with tc.tile_pool(name="mlp", bufs=1) as mpool, \
                     tc.tile_pool(name="wmlp", bufs=2) as mwpool:
                    h1 = mpool.tile([P, FK, SP], F32R, tag="h1")
                    for blk in range(8):
                        wb = mwpool.tile([P, KK, 512], F32R, tag="wmlp")
                        nc.sync.dma_start(wb[:], w1_r[:, :, blk * 512:(blk + 1) * 512])
                        for mi in range(4):
                            m = blk * 4 + mi
                            for (q0, qn) in QC:
                                ps = psA.tile([P, 512], F32, tag="pA")
                                for kk in range(KK):
                                    nc.tensor.matmul(
                                        ps[:, :qn],
                                        wb[:, kk, mi * P:(mi + 1) * P],
                                        xn2_fm[:, kk, q0:q0 + qn],
                                        start=(kk == 0), stop=(kk == KK - 1))
                                nc.scalar.activation(
                                    h1[:, m, q0:q0 + qn], ps[:, :qn],
                                    _GELU, bias=b1_sb[:, m:m + 1])
                    mlp_fm = mpool.tile([P, KK, SP], F32, tag="mlp_fm")
                    for m in range(KK):
                        wb = mwpool.tile([P, FK, P], F32R, tag="wmlp")
                        nc.sync.dma_start(wb[:], w2_r[:, :, m * P:(m + 1) * P])
                        for (q0, qn) in QC:
                            ps = psA.tile([P, 512], F32, tag="pA")
                            for kk2 in range(FK):
                                nc.tensor.matmul(
                                    ps[:, :qn], wb[:, kk2],
                                    h1[:, kk2, q0:q0 + qn],
                                    start=(kk2 == 0), stop=(kk2 == FK - 1))
                            nc.vector.tensor_scalar_add(
                                mlp_fm[:, m, q0:q0 + qn], ps[:, :qn],
                                b2_sb[:, m:m + 1])
                    # final: transpose back to token-major, add residual, store
                    for ti, (t0, pt) in enumerate(TT):
                        rp = min(pt, S - t0)   # skip the pad token on store
                        for kk in range(KK):
                            ps = psA.tile([P, 512], F32, tag="pA")
                            nc.tensor.transpose(
                                ps[:pt, :P], mlp_fm[:, kk, t0:t0 + pt], ident[:])
                            og = opool.tile([P, P], F32, tag="ostg")
                            nc.vector.scalar_tensor_tensor(
                                og[:pt], ps[:pt, :P], 0.0,
                                x2[:pt, ti, kk * P:(kk + 1) * P], OP.add, OP.add)
                            nc.sync.dma_start(
                                y_d[b, t0:t0 + rp, kk * P:(kk + 1) * P], og[:rp])

    nc.compile()
    return nc


def _get_nc():
    global _NC_CACHE
    if _NC_CACHE is None:
        _NC_CACHE = _build()
    return _NC_CACHE


def kernel(**inputs):
    nc = _get_nc()
    x = np.ascontiguousarray(np.asarray(inputs["x"], dtype=np.float32))
    shared = {
        n: np.ascontiguousarray(np.asarray(inputs[n], dtype=np.float32))
        for n in WEIGHT_NAMES
    }
    in_maps = []
    for i in range(NCORES):
        m = dict(shared)
        m["x"] = np.ascontiguousarray(x[i * BL:(i + 1) * BL])
        in_maps.append(m)
    res = bass_utils.run_bass_kernel_spmd(nc, in_maps, core_ids=list(range(NCORES)))
    y = np.concatenate([res.results[i]["y"] for i in range(NCORES)], axis=0)
    return y.astype(np.float32)


# revision 50
# speedup vs baseline: 10673.2297x; 10673.2297x over previous
"""Trainium2 Bass kernel for a dense transformer block (pre-LN attention + GELU MLP).

Strategy: data-parallel over batch across 8 NeuronCores (2 batches/core, no
collectives).  Per core: token-major residual stream with feature-major
activations for matmuls (PE-transpose at the two LayerNorms), fp32r matmuls
(full PE rate), softmax without max-subtraction (scores are O(1) bounded by
construction), PV matmul with a ones-column on V to produce row-sums for free.
"""

import numpy as np

import concourse.bass as bass
import concourse.mybir as mybir
import concourse.tile as tile
from concourse import bacc, bass_utils
from concourse.masks import make_identity

# Problem shape (hardcoded per spec nn_Block_58652073394865)
B, S, D, H, F = 16, 577, 1024, 16, 4096
DH = D // H
NCORES = 8
BL = B // NCORES        # batches per core
P = 128
KK = D // P             # 8 chunks of the model dim
FK = F // P             # 32 chunks of the mlp dim
EPS = 1e-6

# fp32r matmuls require even free-dim counts, so pad tokens 577 -> 578 (one
# zeroed pad token) and use even, overlapping moving-token chunks.
SP = 578
TT = [(0, 128), (128, 128), (256, 128), (384, 128), (512, 66)]   # token tiles (incl pad)
QC = [(0, 290), (288, 290)]                                      # moving-token chunks (even, >=256)
DC = [(0, 512), (512, 512)]                                      # model-dim 512 chunks
VS = 66                                                          # per-head stride in v (64 v + 1 ones + 1 pad)

F32 = mybir.dt.float32
F32R = mybir.dt.float32r
AF = mybir.ActivationFunctionType
OP = mybir.AluOpType

WEIGHT_NAMES = [
    "ln1_g", "ln1_b", "wq", "bq", "wk", "bk", "wv", "bv", "wo", "bo",
    "ln2_g", "ln2_b", "w1", "b1", "w2", "b2",
]

_NC_CACHE = None
# CoreSim doesn't implement the Gelu LUT; tests may swap this for AF.Tanh
_GELU = AF.Gelu


def _build():
    nc = bacc.Bacc("TRN2", target_bir_lowering=False, debug=False,
                   num_devices=NCORES)

    x_d = nc.dram_tensor("x", [BL, S, D], F32, kind="ExternalInput").ap()
    y_d = nc.dram_tensor("y", [BL, S, D], F32, kind="ExternalOutput").ap()
    # weights consumed by matmuls -> declare fp32r (same bits as fp32)
    wq_d = nc.dram_tensor("wq", [D, D], F32R, kind="ExternalInput").ap()
    wk_d = nc.dram_tensor("wk", [D, D], F32R, kind="ExternalInput").ap()
    wv_d = nc.dram_tensor("wv", [D, D], F32R, kind="ExternalInput").ap()
    wo_d = nc.dram_tensor("wo", [D, D], F32R, kind="ExternalInput").ap()
    w1_d = nc.dram_tensor("w1", [D, F], F32R, kind="ExternalInput").ap()
    w2_d = nc.dram_tensor("w2", [F, D], F32R, kind="ExternalInput").ap()
    bv_d = nc.dram_tensor("bv", [D], F32R, kind="ExternalInput").ap()   # folded via K=1 matmul
    bo_d = nc.dram_tensor("bo", [D], F32R, kind="ExternalInput").ap()   # folded via K=1 matmul
    bq_d = nc.dram_tensor("bq", [D], F32, kind="ExternalInput").ap()
    bk_d = nc.dram_tensor("bk", [D], F32, kind="ExternalInput").ap()
    b1_d = nc.dram_tensor("b1", [F], F32, kind="ExternalInput").ap()
    b2_d = nc.dram_tensor("b2", [D], F32, kind="ExternalInput").ap()
    g1_d = nc.dram_tensor("ln1_g", [D], F32, kind="ExternalInput").ap()
    gb1_d = nc.dram_tensor("ln1_b", [D], F32, kind="ExternalInput").ap()
    g2_d = nc.dram_tensor("ln2_g", [D], F32, kind="ExternalInput").ap()
    gb2_d = nc.dram_tensor("ln2_b", [D], F32, kind="ExternalInput").ap()

    wq_r = wq_d.rearrange("(ko p) d -> p ko d", p=P)
    wk_r = wk_d.rearrange("(ko p) d -> p ko d", p=P)
    wv_r = wv_d.rearrange("(ko p) d -> p ko d", p=P)
    wo_r = wo_d.rearrange("(ko p) d -> p ko d", p=P)
    w1_r = w1_d.rearrange("(ko p) d -> p ko d", p=P)
    w2_r = w2_d.rearrange("(ko p) d -> p ko d", p=P)

    with tile.TileContext(nc) as tc:
        with tc.tile_pool(name="const", bufs=1) as cpool, \
             tc.tile_pool(name="resid", bufs=2) as rpool, \
             tc.tile_pool(name="fmbuf", bufs=1) as fmpool, \
             tc.tile_pool(name="ostg", bufs=4) as opool, \
             tc.tile_pool(name="lnp", bufs=2) as lnpool, \
             tc.tile_pool(name="psA", bufs=4, space="PSUM") as psA:

            # ---- constants / small params ----
            # tiles pad to 4KB/partition: pack the small params into few tiles
            cA = cpool.tile([P, 7 * KK + FK], F32, tag="cA")
            bq_sb = cA[:, 0:KK]
            bk_sb = cA[:, KK:2 * KK]
            b2_sb = cA[:, 2 * KK:3 * KK]
            g1_sb = cA[:, 3 * KK:4 * KK]
            gb1_sb = cA[:, 4 * KK:5 * KK]
            g2_sb = cA[:, 5 * KK:6 * KK]
            gb2_sb = cA[:, 6 * KK:7 * KK]
            b1_sb = cA[:, 7 * KK:7 * KK + FK]
            nc.sync.dma_start(bq_sb, bq_d.rearrange("(m p) -> p m", p=P))
            nc.sync.dma_start(bk_sb, bk_d.rearrange("(m p) -> p m", p=P))
            nc.sync.dma_start(b2_sb, b2_d.rearrange("(m p) -> p m", p=P))
            nc.sync.dma_start(g1_sb, g1_d.rearrange("(c p) -> p c", p=P))
            nc.sync.dma_start(gb1_sb, gb1_d.rearrange("(c p) -> p c", p=P))
            nc.sync.dma_start(g2_sb, g2_d.rearrange("(c p) -> p c", p=P))
            nc.sync.dma_start(gb2_sb, gb2_d.rearrange("(c p) -> p c", p=P))
            nc.sync.dma_start(b1_sb, b1_d.rearrange("(m p) -> p m", p=P))

            cB = cpool.tile([P, P + 2], F32, tag="cB")
            ident = cB[:, 0:P]
            epsap = cB[:, P:P + 1]
            onec_f = cB[:, P + 1:P + 2]
            make_identity(nc, ident)
            nc.vector.memset(epsap, EPS)
            nc.vector.memset(onec_f, 1.0)

            ones_f = cpool.tile([1, P], F32, tag="ones_f")
            nc.vector.memset(ones_f[:], 1.0)
            cD = cpool.tile([1, P + 2 * D], F32R, tag="cD")
            ones_r = cD[:, 0:P]
            t_bo = cD[:, P:P + D]
            t_bv = cD[:, P + D:P + 2 * D]
            nc.vector.tensor_copy(ones_r, ones_f[:])
            nc.sync.dma_start(t_bo, bo_d[None, :])
            nc.sync.dma_start(t_bv, bv_d[None, :])

            # token-major layernorm -> feature-major normalized output
            def ln_new_stats(ln_pool):
                stats = ln_pool.tile([P, 20], F32, tag="stats")
                # last token tile covers only 66 partitions; keep the rest defined
                nc.vector.memset(stats[:, 0:5], 0.0)
                nc.vector.memset(stats[:, 5:10], 1.0)
                return stats

            def ln_tile_stats(ln_pool, stats, src, ti, pt):
                negmu = stats[:, 0:5]
                varD = stats[:, 5:10]
                nc.vector.tensor_reduce(
                    negmu[:pt, ti:ti + 1], src[:pt, ti],
                    mybir.AxisListType.X, OP.add)
                nc.vector.tensor_scalar_mul(
                    negmu[:pt, ti:ti + 1], negmu[:pt, ti:ti + 1], -1.0 / D)
                scr = ln_pool.tile([P, D], F32, tag="xn_tm")
                nc.scalar.activation(
                    scr[:pt], src[:pt, ti], AF.Square,
                    bias=negmu[:pt, ti:ti + 1], accum_out=varD[:pt, ti:ti + 1])

            def layer_norm_fm(ln_pool, src, g_sb, gb_sb, dst_fm, stats=None):
                if stats is None:
                    stats = ln_new_stats(ln_pool)
                    for ti, (t0, pt) in enumerate(TT):
                        ln_tile_stats(ln_pool, stats, src, ti, pt)
                negmu = stats[:, 0:5]
                varD = stats[:, 5:10]
                sig = stats[:, 10:15]
                rsig = stats[:, 15:20]
                # split so tiles 0-3 (whose stats land first) unblock their
                # normalize/transpose before the producer's last tile arrives
                nc.scalar.activation(sig[:, 0:4], varD[:, 0:4], AF.Sqrt,
                                     scale=1.0 / D, bias=epsap[:])
                nc.vector.reciprocal(rsig[:, 0:4], sig[:, 0:4])
                nc.scalar.activation(sig[:, 4:5], varD[:, 4:5], AF.Sqrt,
                                     scale=1.0 / D, bias=epsap[:])
                nc.vector.reciprocal(rsig[:, 4:5], sig[:, 4:5])
                for ti, (t0, pt) in enumerate(TT):
                    xn = ln_pool.tile([P, D], F32, tag="xn_tm")
                    nc.vector.tensor_scalar(
                        xn[:pt], src[:pt, ti],
                        negmu[:pt, ti:ti + 1], rsig[:pt, ti:ti + 1],
                        OP.add, OP.mult)
                    for kk in range(KK):
                        pst = psA.tile([P, 512], F32, tag="pA")
                        nc.tensor.transpose(
                            pst[:, :pt], xn[:pt, kk * P:(kk + 1) * P],
                            ident[:pt, :pt])
                        nc.vector.scalar_tensor_tensor(
                            dst_fm[:, kk, t0:t0 + pt], pst[:, :pt],
                            g_sb[:, kk:kk + 1],
                            gb_sb[:, kk:kk + 1].to_broadcast((P, pt)),
                            OP.mult, OP.add)

            for b in range(BL):
                xn_fm = fmpool.tile([P, KK, SP], F32R, tag="xn_fm")
                xb = rpool.tile([P, 5, D], F32, tag="resid")

                # ---- stage A: load x (token-major); zero the pad token row ----
                # (engine start-partition must be a multiple of 32: zero 64..127
                # first, then the DMA rewrites the real rows 0..64)
                nc.vector.memset(xb[64:, 4, :], 0.0)
                for ti, (t0, pt) in enumerate(TT):
                    rp = min(pt, S - t0)   # real (non-pad) tokens in this tile
                    nc.sync.dma_start(xb[:rp, ti], x_d[b, t0:t0 + rp, :])

                # ---- stage B: LN1 -> xn_fm ----
                layer_norm_fm(lnpool, xb, g1_sb, gb1_sb, xn_fm)

                with tc.tile_pool(name="attn", bufs=1) as apool, \
                     tc.tile_pool(name="wblk", bufs=2) as wpool:
                    q_fm = apool.tile([P, KK, SP], F32R, tag="q")
                    k_fm = apool.tile([P, KK, SP], F32R, tag="k")
                    v_sb = apool.tile([P, 5, H * VS], F32R, tag="v")
                    ctx_fm = apool.tile([P, KK, SP], F32R, tag="ctx")

                    # col 64 of each head's stride-66 group = 1 (rowsum trick),
                    # col 65 = 0 (fp32r even-M pad).  The pad token's whole v
                    # row (tile 4, partition 65) must be zero: zero partitions
                    # 64.. first, later writes refill only the real rows.
                    v_hc = v_sb[:].rearrange("p t (h c) -> p t h c", c=VS)
                    # memset can't target fp32r; zero via a uint32 view
                    nc.vector.memset(v_hc[64:, 4:5].bitcast(mybir.dt.uint32), 0)
                    nc.vector.memset(v_hc[:, :, :, 65:66].bitcast(mybir.dt.uint32), 0)
                    nc.vector.tensor_copy(
                        v_hc[:, 0:4, :, 64:65],
                        onec_f[:, :, None, None].to_broadcast((P, 4, H, 1)))
                    nc.vector.tensor_copy(
                        v_hc[:65, 4:5, :, 64:65],
                        onec_f[:65, :, None, None].to_broadcast((65, 1, H, 1)))

                    # ---- stage C/D interleaved: projections + attention ----
                    # blk covers q/k m-tiles 4*blk..4*blk+3 and v heads
                    # 8*blk..8*blk+7 == attention heads 8*blk..8*blk+7, so each
                    # half's projections feed its attention while the NEXT
                    # half's projection matmuls fill the exp-bound PE idle.
                    def emit_qk(blk):
                        for w_r, bias_sb, dst in ((wq_r, bq_sb, q_fm), (wk_r, bk_sb, k_fm)):
                            wb = wpool.tile([P, KK, 512], F32R, tag="wblk")
                            nc.sync.dma_start(wb[:], w_r[:, :, blk * 512:(blk + 1) * 512])
                            for mi in range(4):
                                m = blk * 4 + mi
                                for (q0, qn) in QC:
                                    ps = psA.tile([P, 512], F32, tag="pA")
                                    for kk in range(KK):
                                        nc.tensor.matmul(
                                            ps[:, :qn],
                                            wb[:, kk, mi * P:(mi + 1) * P],
                                            xn_fm[:, kk, q0:q0 + qn],
                                            start=(kk == 0), stop=(kk == KK - 1))
                                    nc.scalar.activation(
                                        dst[:, m, q0:q0 + qn], ps[:, :qn],
                                        AF.Identity, bias=bias_sb[:, m:m + 1])

                    def emit_v(ci):
                        c0, cn = DC[ci]
                        wb = wpool.tile([P, KK, 512], F32R, tag="wblk")
                        nc.sync.dma_start(wb[:], wv_r[:, :, c0:c0 + cn])
                        for ti, (t0, pt) in enumerate(TT):
                            ps = psA.tile([P, 512], F32, tag="pA")
                            for kk in range(KK):
                                nc.tensor.matmul(
                                    ps[:pt], xn_fm[:, kk, t0:t0 + pt],
                                    wb[:, kk, :], start=(kk == 0), stop=False)
                            nc.tensor.matmul(
                                ps[:pt], ones_r[:, :pt], t_bv[:, c0:c0 + cn],
                                start=False, stop=True)
                            rp = min(pt, S - t0)
                            nc.vector.tensor_copy(
                                v_sb[:rp, ti].rearrange("p (h c) -> p h c", c=VS)[:, ci * 8:(ci + 1) * 8, 0:64],
                                ps[:rp, :cn].rearrange("p (h c) -> p h c", c=64))

                    def emit_attn(h):
                        hrow = (h % 2) * 64
                        kkh = h // 2
                        for qi, (q0, qn) in enumerate(QC):
                            es = apool.tile([P, 5, qn], F32R, tag=f"es{qi}")
                            # pair the 5 score tiles into 2-bank psum groups so
                            # each Exp covers 2 tiles (halves the per-op cost)
                            for pair in ((0, 1), (2, 3), (4,)):
                                pg = psA.tile([P, 2, 512], F32, tag="pS", bufs=2)
                                for j, kt in enumerate(pair):
                                    t0, ptk = TT[kt]
                                    nc.tensor.matmul(
                                        pg[:ptk, j, :qn],
                                        k_fm[hrow:hrow + 64, kkh, t0:t0 + ptk],
                                        q_fm[hrow:hrow + 64, kkh, q0:q0 + qn],
                                        start=True, stop=True)
                                npair = len(pair)
                                prow = TT[pair[0]][1]   # 128 for full pairs, 66 for (4,)
                                nc.scalar.activation(
                                    es[:prow, pair[0]:pair[0] + npair, :],
                                    pg[:prow, :npair, :qn],
                                    AF.Exp, scale=1.0 / np.sqrt(DH))
                            pc = psA.tile([VS, 512], F32, tag="pA")
                            for kt, (t0, ptk) in enumerate(TT):
                                nc.tensor.matmul(
                                    pc[:, :qn],
                                    v_sb[:ptk, kt, h * VS:(h + 1) * VS],
                                    es[:ptk, kt, :],
                                    start=(kt == 0), stop=(kt == 4))
                            rc = apool.tile([1, 290], F32, tag="rc", bufs=2)
                            nc.vector.reciprocal(rc[:, :qn], pc[64:65, :qn])
                            rb = apool.tile([64, 290], F32, tag="rb", bufs=2)
                            nc.gpsimd.partition_broadcast(rb[:, :qn], rc[:, :qn])
                            nc.vector.tensor_tensor(
                                ctx_fm[hrow:hrow + 64, kkh, q0:q0 + qn],
                                pc[0:64, :qn], rb[:, :qn], OP.mult)

                    emit_qk(0)
                    emit_v(0)
                    for h in range(8):
                        emit_attn(h)
                    emit_qk(1)
                    emit_v(1)
                    for h in range(8, H):
                        emit_attn(h)

                    # ---- stage E: output projection + residual -> x2 ----
                    x2 = rpool.tile([P, 5, D], F32, tag="resid")
                    stats2 = ln_new_stats(lnpool)
                    for ci, (c0, cn) in enumerate(DC):
                        wb = wpool.tile([P, KK, 512], F32R, tag="wblk")
                        nc.sync.dma_start(wb[:], wo_r[:, :, c0:c0 + cn])
                        for ti, (t0, pt) in enumerate(TT):
                            ps = psA.tile([P, 512], F32, tag="pA")
                            for kk in range(KK):
                                nc.tensor.matmul(
                                    ps[:pt], ctx_fm[:, kk, t0:t0 + pt],
                                    wb[:, kk, :], start=(kk == 0), stop=False)
                            nc.tensor.matmul(
                                ps[:pt], ones_r[:, :pt], t_bo[:, c0:c0 + cn],
                                start=False, stop=True)
                            nc.vector.scalar_tensor_tensor(
                                x2[:pt, ti, c0:c0 + cn], ps[:pt], 0.0,
                                xb[:pt, ti, c0:c0 + cn], OP.add, OP.add)
                            if ci == len(DC) - 1:
                                # x2 tile complete: fold its LN2 stats in now
                                ln_tile_stats(lnpool, stats2, x2, ti, pt)

                # ---- stage F: LN2 -> xn_fm (rotates) ----
                xn2_fm = fmpool.tile([P, KK, SP], F32R, tag="xn_fm")
                layer_norm_fm(lnpool, x2, g2_sb, gb2_sb, xn2_fm, stats=stats2)

                # ---- stage G: MLP ----
                with tc.tile_pool(name="mlp", bufs=1) as mpool, \
                     tc.tile_pool(name="wmlp", bufs=2) as mwpool:
                    h1 = mpool.tile([P, FK, SP], F32R, tag="h1")
                    for blk in range(8):
                        wb = mwpool.tile([P, KK, 512], F32R, tag="wmlp")
                        nc.sync.dma_start(wb[:], w1_r[:, :, blk * 512:(blk + 1) * 512])
                        for mi in range(4):
                            m = blk * 4 + mi
                            for (q0, qn) in QC:
                                ps = psA.tile([P, 512], F32, tag="pA")
                                for kk in range(KK):
                                    nc.tensor.matmul(
                                        ps[:, :qn],
                                        wb[:, kk, mi * P:(mi + 1) * P],
                                        xn2_fm[:, kk, q0:q0 + qn],
                                        start=(kk == 0), stop=(kk == KK - 1))
                                nc.scalar.activation(
                                    h1[:, m, q0:q0 + qn], ps[:, :qn],
                                    _GELU, bias=b1_sb[:, m:m + 1])
                    mlp_fm = mpool.tile([P, KK, SP], F32, tag="mlp_fm")
                    for m in range(KK):
                        wb = mwpool.tile([P, FK, P], F32R, tag="wmlp")
                        nc.sync.dma_start(wb[:], w2_r[:, :, m * P:(m + 1) * P])
                        for (q0, qn) in QC:
                            ps = psA.tile([P, 512], F32, tag="pA")
                            for kk2 in range(FK):
                                nc.tensor.matmul(
                                    ps[:, :qn], wb[:, kk2],
                                    h1[:, kk2, q0:q0 + qn],
                                    start=(kk2 == 0), stop=(kk2 == FK - 1))
                            nc.vector.tensor_scalar_add(
                                mlp_fm[:, m, q0:q0 + qn], ps[:, :qn],
                                b2_sb[:, m:m + 1])
                        # this m's feature rows are complete: transpose back to
                        # token-major, add residual, store (interleaves with the
                        # next m's w2 matmuls)
                        for ti, (t0, pt) in enumerate(TT):
                            rp = min(pt, S - t0)   # skip the pad token on store
                            ps = psA.tile([P, 512], F32, tag="pA")
                            nc.tensor.transpose(
                                ps[:pt, :P], mlp_fm[:, m, t0:t0 + pt], ident[:])
                            og = opool.tile([P, P], F32, tag="ostg", bufs=6)
                            nc.vector.scalar_tensor_tensor(
                                og[:pt], ps[:pt, :P], 0.0,
                                x2[:pt, ti, m * P:(m + 1) * P], OP.add, OP.add)
                            nc.sync.dma_start(
                                y_d[b, t0:t0 + rp, m * P:(m + 1) * P], og[:rp])

    nc.compile()
    return nc


def _get_nc():
    global _NC_CACHE
    if _NC_CACHE is None:
        _NC_CACHE = _build()
    return _NC_CACHE


def kernel(**inputs):
    nc = _get_nc()
    x = np.ascontiguousarray(np.asarray(inputs["x"], dtype=np.float32))
    shared = {
        n: np.ascontiguousarray(np.asarray(inputs[n], dtype=np.float32))
        for n in WEIGHT_NAMES
    }
    in_maps = []
    for i in range(NCORES):
        m = dict(shared)
        m["x"] = np.ascontiguousarray(x[i * BL:(i + 1) * BL])
        in_maps.append(m)
    res = bass_utils.run_bass_kernel_spmd(nc, in_maps, core_ids=list(range(NCORES)))
    y = np.concatenate([res.results[i]["y"] for i in range(NCORES)], axis=0)
    return y.astype(np.float32)


# revision 55
# speedup vs baseline: 10739.9531x; 1.0063x over previous
"""Trainium2 Bass kernel for a dense transformer block (pre-LN attention + GELU MLP).

Strategy: data-parallel over batch across 8 NeuronCores (2 batches/core, no
collectives).  Per core: token-major residual stream with feature-major
activations for matmuls (PE-transpose at the two LayerNorms), fp32r matmuls
(full PE rate), softmax without max-subtraction (scores are O(1) bounded by
construction), PV matmul with a ones-column on V to produce row-sums for free.
"""

import numpy as np

import concourse.bass as bass
import concourse.mybir as mybir
import concourse.tile as tile
from concourse import bacc, bass_utils
from concourse.masks import make_identity

# Problem shape (hardcoded per spec nn_Block_58652073394865)
B, S, D, H, F = 16, 577, 1024, 16, 4096
DH = D // H
NCORES = 8
BL = B // NCORES        # batches per core
P = 128
KK = D // P             # 8 chunks of the model dim
FK = F // P             # 32 chunks of the mlp dim
EPS = 1e-6

# fp32r matmuls require even free-dim counts, so pad tokens 577 -> 578 (one
# zeroed pad token) and use even, overlapping moving-token chunks.
SP = 578
TT = [(0, 128), (128, 128), (256, 128), (384, 128), (512, 66)]   # token tiles (incl pad)
QC = [(0, 290), (288, 290)]                                      # moving-token chunks (even, >=256)
DC = [(0, 512), (512, 512)]                                      # model-dim 512 chunks
VS = 66                                                          # per-head stride in v (64 v + 1 ones + 1 pad)

F32 = mybir.dt.float32
F32R = mybir.dt.float32r
AF = mybir.ActivationFunctionType
OP = mybir.AluOpType

WEIGHT_NAMES = [
    "ln1_g", "ln1_b", "wq", "bq", "wk", "bk", "wv", "bv", "wo", "bo",
    "ln2_g", "ln2_b", "w1", "b1", "w2", "b2",
]

_NC_CACHE = None
# CoreSim doesn't implement the Gelu LUT; tests may swap this for AF.Tanh
_GELU = AF.Gelu


def _build():
    nc = bacc.Bacc("TRN2", target_bir_lowering=False, debug=False,
                   num_devices=NCORES)

    x_d = nc.dram_tensor("x", [BL, S, D], F32, kind="ExternalInput").ap()
    y_d = nc.dram_tensor("y", [BL, S, D], F32, kind="ExternalOutput").ap()
    # weights consumed by matmuls -> declare fp32r (same bits as fp32)
    wq_d = nc.dram_tensor("wq", [D, D], F32R, kind="ExternalInput").ap()
    wk_d = nc.dram_tensor("wk", [D, D], F32R, kind="ExternalInput").ap()
    wv_d = nc.dram_tensor("wv", [D, D], F32R, kind="ExternalInput").ap()
    wo_d = nc.dram_tensor("wo", [D, D], F32R, kind="ExternalInput").ap()
    w1_d = nc.dram_tensor("w1", [D, F], F32R, kind="ExternalInput").ap()
    w2_d = nc.dram_tensor("w2", [F, D], F32R, kind="ExternalInput").ap()
    bv_d = nc.dram_tensor("bv", [D], F32R, kind="ExternalInput").ap()   # folded via K=1 matmul
    bo_d = nc.dram_tensor("bo", [D], F32R, kind="ExternalInput").ap()   # folded via K=1 matmul
    bq_d = nc.dram_tensor("bq", [D], F32, kind="ExternalInput").ap()
    bk_d = nc.dram_tensor("bk", [D], F32, kind="ExternalInput").ap()
    b1_d = nc.dram_tensor("b1", [F], F32, kind="ExternalInput").ap()
    b2_d = nc.dram_tensor("b2", [D], F32, kind="ExternalInput").ap()
    g1_d = nc.dram_tensor("ln1_g", [D], F32, kind="ExternalInput").ap()
    gb1_d = nc.dram_tensor("ln1_b", [D], F32, kind="ExternalInput").ap()
    g2_d = nc.dram_tensor("ln2_g", [D], F32, kind="ExternalInput").ap()
    gb2_d = nc.dram_tensor("ln2_b", [D], F32, kind="ExternalInput").ap()

    wq_r = wq_d.rearrange("(ko p) d -> p ko d", p=P)
    wk_r = wk_d.rearrange("(ko p) d -> p ko d", p=P)
    wv_r = wv_d.rearrange("(ko p) d -> p ko d", p=P)
    wo_r = wo_d.rearrange("(ko p) d -> p ko d", p=P)
    w1_r = w1_d.rearrange("(ko p) d -> p ko d", p=P)
    w2_r = w2_d.rearrange("(ko p) d -> p ko d", p=P)

    with tile.TileContext(nc) as tc:
        with tc.tile_pool(name="const", bufs=1) as cpool, \
             tc.tile_pool(name="resid", bufs=2) as rpool, \
             tc.tile_pool(name="fmbuf", bufs=1) as fmpool, \
             tc.tile_pool(name="ostg", bufs=4) as opool, \
             tc.tile_pool(name="lnp", bufs=2) as lnpool, \
             tc.tile_pool(name="psA", bufs=4, space="PSUM") as psA:

            # ---- constants / small params ----
            # tiles pad to 4KB/partition: pack the small params into few tiles
            cA = cpool.tile([P, 7 * KK + FK], F32, tag="cA")
            bq_sb = cA[:, 0:KK]
            bk_sb = cA[:, KK:2 * KK]
            b2_sb = cA[:, 2 * KK:3 * KK]
            g1_sb = cA[:, 3 * KK:4 * KK]
            gb1_sb = cA[:, 4 * KK:5 * KK]
            g2_sb = cA[:, 5 * KK:6 * KK]
            gb2_sb = cA[:, 6 * KK:7 * KK]
            b1_sb = cA[:, 7 * KK:7 * KK + FK]
            nc.sync.dma_start(bq_sb, bq_d.rearrange("(m p) -> p m", p=P))
            nc.sync.dma_start(bk_sb, bk_d.rearrange("(m p) -> p m", p=P))
            nc.sync.dma_start(b2_sb, b2_d.rearrange("(m p) -> p m", p=P))
            nc.sync.dma_start(g1_sb, g1_d.rearrange("(c p) -> p c", p=P))
            nc.sync.dma_start(gb1_sb, gb1_d.rearrange("(c p) -> p c", p=P))
            nc.sync.dma_start(g2_sb, g2_d.rearrange("(c p) -> p c", p=P))
            nc.sync.dma_start(gb2_sb, gb2_d.rearrange("(c p) -> p c", p=P))
            nc.sync.dma_start(b1_sb, b1_d.rearrange("(m p) -> p m", p=P))

            cB = cpool.tile([P, P + 2], F32, tag="cB")
            ident = cB[:, 0:P]
            epsap = cB[:, P:P + 1]
            onec_f = cB[:, P + 1:P + 2]
            make_identity(nc, ident)
            nc.vector.memset(epsap, EPS)
            nc.vector.memset(onec_f, 1.0)

            ident_r = cpool.tile([P, P], F32R, tag="ident_r")
            nc.vector.tensor_copy(ident_r[:], ident)

            ones_f = cpool.tile([1, P], F32, tag="ones_f")
            nc.vector.memset(ones_f[:], 1.0)
            cD = cpool.tile([1, P + 2 * D], F32R, tag="cD")
            ones_r = cD[:, 0:P]
            t_bo = cD[:, P:P + D]
            t_bv = cD[:, P + D:P + 2 * D]
            nc.vector.tensor_copy(ones_r, ones_f[:])
            nc.sync.dma_start(t_bo, bo_d[None, :])
            nc.sync.dma_start(t_bv, bv_d[None, :])

            # token-major layernorm -> feature-major normalized output
            def ln_new_stats(ln_pool):
                stats = ln_pool.tile([P, 20], F32, tag="stats")
                # last token tile covers only 66 partitions; keep the rest defined
                nc.vector.memset(stats[:, 0:5], 0.0)
                nc.vector.memset(stats[:, 5:10], 1.0)
                return stats

            def ln_tile_stats(ln_pool, stats, src, ti, pt):
                negmu = stats[:, 0:5]
                varD = stats[:, 5:10]
                nc.vector.tensor_reduce(
                    negmu[:pt, ti:ti + 1], src[:pt, ti],
                    mybir.AxisListType.X, OP.add)
                nc.vector.tensor_scalar_mul(
                    negmu[:pt, ti:ti + 1], negmu[:pt, ti:ti + 1], -1.0 / D)
                scr = ln_pool.tile([P, D], F32R, tag="xn_tm", bufs=3)
                nc.scalar.activation(
                    scr[:pt], src[:pt, ti], AF.Square,
                    bias=negmu[:pt, ti:ti + 1], accum_out=varD[:pt, ti:ti + 1])

            def ln_finalize(stats, lo, hi):
                # rsig for tile range [lo, hi)
                nc.scalar.activation(stats[:, 10 + lo:10 + hi],
                                     stats[:, 5 + lo:5 + hi], AF.Sqrt,
                                     scale=1.0 / D, bias=epsap[:])
                nc.vector.reciprocal(stats[:, 15 + lo:15 + hi],
                                     stats[:, 10 + lo:10 + hi])

            def ln_apply_tiles(ln_pool, stats, src, g_sb, gb_sb, dst_fm, tis):
                negmu = stats[:, 0:5]
                rsig = stats[:, 15:20]
                for ti in tis:
                    t0, pt = TT[ti]
                    xn = ln_pool.tile([P, D], F32R, tag="xn_tm", bufs=3)
                    nc.vector.tensor_scalar(
                        xn[:pt], src[:pt, ti],
                        negmu[:pt, ti:ti + 1], rsig[:pt, ti:ti + 1],
                        OP.add, OP.mult)
                    for kk in range(KK):
                        pst = psA.tile([P, 512], F32R, tag="pA")
                        nc.tensor.transpose(
                            pst[:, :pt], xn[:pt, kk * P:(kk + 1) * P],
                            ident_r[:pt, :pt])
                        nc.vector.scalar_tensor_tensor(
                            dst_fm[:, kk, t0:t0 + pt], pst[:, :pt],
                            g_sb[:, kk:kk + 1],
                            gb_sb[:, kk:kk + 1].to_broadcast((P, pt)),
                            OP.mult, OP.add)

            def layer_norm_fm(ln_pool, src, g_sb, gb_sb, dst_fm):
                stats = ln_new_stats(ln_pool)
                for ti, (t0, pt) in enumerate(TT):
                    ln_tile_stats(ln_pool, stats, src, ti, pt)
                ln_finalize(stats, 0, 4)
                ln_apply_tiles(ln_pool, stats, src, g_sb, gb_sb, dst_fm, (0, 1, 2, 3))
                ln_finalize(stats, 4, 5)
                ln_apply_tiles(ln_pool, stats, src, g_sb, gb_sb, dst_fm, (4,))

            for b in range(BL):
                xn_fm = fmpool.tile([P, KK, SP], F32R, tag="xn_fm")
                xb = rpool.tile([P, 5, D], F32, tag="resid")

                # ---- stage A: load x (token-major); zero the pad token row ----
                # (engine start-partition must be a multiple of 32: zero 64..127
                # first, then the DMA rewrites the real rows 0..64)
                nc.vector.memset(xb[64:, 4, :], 0.0)
                for ti, (t0, pt) in enumerate(TT):
                    rp = min(pt, S - t0)   # real (non-pad) tokens in this tile
                    nc.sync.dma_start(xb[:rp, ti], x_d[b, t0:t0 + rp, :])

                # ---- stage B: LN1 -> xn_fm ----
                layer_norm_fm(lnpool, xb, g1_sb, gb1_sb, xn_fm)

                with tc.tile_pool(name="attn", bufs=1) as apool, \
                     tc.tile_pool(name="wblk", bufs=2) as wpool:
                    q_fm = apool.tile([P, KK, SP], F32R, tag="q")
                    k_fm = apool.tile([P, KK, SP], F32R, tag="k")
                    v_sb = apool.tile([P, 5, H * VS], F32R, tag="v")
                    ctx_fm = apool.tile([P, KK, SP], F32R, tag="ctx")

                    # col 64 of each head's stride-66 group = 1 (rowsum trick),
                    # col 65 = 0 (fp32r even-M pad).  The pad token's whole v
                    # row (tile 4, partition 65) must be zero: zero partitions
                    # 64.. first, later writes refill only the real rows.
                    v_hc = v_sb[:].rearrange("p t (h c) -> p t h c", c=VS)
                    # memset can't target fp32r; zero via a uint32 view
                    nc.vector.memset(v_hc[64:, 4:5].bitcast(mybir.dt.uint32), 0)
                    nc.vector.memset(v_hc[:, :, :, 65:66].bitcast(mybir.dt.uint32), 0)
                    nc.vector.tensor_copy(
                        v_hc[:, 0:4, :, 64:65],
                        onec_f[:, :, None, None].to_broadcast((P, 4, H, 1)))
                    nc.vector.tensor_copy(
                        v_hc[:65, 4:5, :, 64:65],
                        onec_f[:65, :, None, None].to_broadcast((65, 1, H, 1)))

                    # ---- stage C/D interleaved: projections + attention ----
                    # blk covers q/k m-tiles 4*blk..4*blk+3 and v heads
                    # 8*blk..8*blk+7 == attention heads 8*blk..8*blk+7, so each
                    # half's projections feed its attention while the NEXT
                    # half's projection matmuls fill the exp-bound PE idle.
                    def emit_qk(blk):
                        for w_r, bias_sb, dst in ((wq_r, bq_sb, q_fm), (wk_r, bk_sb, k_fm)):
                            wb = wpool.tile([P, KK, 512], F32R, tag="wblk")
                            nc.sync.dma_start(wb[:], w_r[:, :, blk * 512:(blk + 1) * 512])
                            for mi in range(4):
                                m = blk * 4 + mi
                                for (q0, qn) in QC:
                                    ps = psA.tile([P, 512], F32, tag="pA")
                                    for kk in range(KK):
                                        nc.tensor.matmul(
                                            ps[:, :qn],
                                            wb[:, kk, mi * P:(mi + 1) * P],
                                            xn_fm[:, kk, q0:q0 + qn],
                                            start=(kk == 0), stop=(kk == KK - 1))
                                    nc.scalar.activation(
                                        dst[:, m, q0:q0 + qn], ps[:, :qn],
                                        AF.Identity, bias=bias_sb[:, m:m + 1])

                    def emit_v(ci):
                        c0, cn = DC[ci]
                        wb = wpool.tile([P, KK, 512], F32R, tag="wblk")
                        nc.sync.dma_start(wb[:], wv_r[:, :, c0:c0 + cn])
                        for ti, (t0, pt) in enumerate(TT):
                            ps = psA.tile([P, 512], F32, tag="pA")
                            for kk in range(KK):
                                nc.tensor.matmul(
                                    ps[:pt], xn_fm[:, kk, t0:t0 + pt],
                                    wb[:, kk, :], start=(kk == 0), stop=False)
                            nc.tensor.matmul(
                                ps[:pt], ones_r[:, :pt], t_bv[:, c0:c0 + cn],
                                start=False, stop=True)
                            rp = min(pt, S - t0)
                            nc.vector.tensor_copy(
                                v_sb[:rp, ti].rearrange("p (h c) -> p h c", c=VS)[:, ci * 8:(ci + 1) * 8, 0:64],
                                ps[:rp, :cn].rearrange("p (h c) -> p h c", c=64))

                    def emit_attn(h):
                        hrow = (h % 2) * 64
                        kkh = h // 2
                        for qi, (q0, qn) in enumerate(QC):
                            es = apool.tile([P, 5, qn], F32R, tag=f"es{qi}")
                            # pair the 5 score tiles into 2-bank psum groups so
                            # each Exp covers 2 tiles (halves the per-op cost)
                            for pair in ((0, 1), (2, 3), (4,)):
                                pg = psA.tile([P, 2, 512], F32, tag="pS", bufs=2)
                                for j, kt in enumerate(pair):
                                    t0, ptk = TT[kt]
                                    nc.tensor.matmul(
                                        pg[:ptk, j, :qn],
                                        k_fm[hrow:hrow + 64, kkh, t0:t0 + ptk],
                                        q_fm[hrow:hrow + 64, kkh, q0:q0 + qn],
                                        start=True, stop=True)
                                npair = len(pair)
                                prow = TT[pair[0]][1]   # 128 for full pairs, 66 for (4,)
                                nc.scalar.activation(
                                    es[:prow, pair[0]:pair[0] + npair, :],
                                    pg[:prow, :npair, :qn],
                                    AF.Exp, scale=1.0 / np.sqrt(DH))
                            pc = psA.tile([VS, 512], F32, tag="pA")
                            for kt, (t0, ptk) in enumerate(TT):
                                nc.tensor.matmul(
                                    pc[:, :qn],
                                    v_sb[:ptk, kt, h * VS:(h + 1) * VS],
                                    es[:ptk, kt, :],
                                    start=(kt == 0), stop=(kt == 4))
                            rc = apool.tile([1, 290], F32, tag="rc", bufs=2)
                            nc.vector.reciprocal(rc[:, :qn], pc[64:65, :qn])
                            rb = apool.tile([64, 290], F32, tag="rb", bufs=2)
                            nc.gpsimd.partition_broadcast(rb[:, :qn], rc[:, :qn])
                            nc.vector.tensor_tensor(
                                ctx_fm[hrow:hrow + 64, kkh, q0:q0 + qn],
                                pc[0:64, :qn], rb[:, :qn], OP.mult)

                    emit_qk(0)
                    emit_v(0)
                    for h in range(8):
                        emit_attn(h)
                    emit_qk(1)
                    emit_v(1)
                    for h in range(8, H):
                        emit_attn(h)

                    # ---- stage E: output projection + residual -> x2,
                    # with LN2 folded in per-tile ----
                    x2 = rpool.tile([P, 5, D], F32, tag="resid")
                    xn2_fm = fmpool.tile([P, KK, SP], F32R, tag="xn_fm")
                    stats2 = ln_new_stats(lnpool)
                    for ci, (c0, cn) in enumerate(DC):
                        wb = wpool.tile([P, KK, 512], F32R, tag="wblk")
                        nc.sync.dma_start(wb[:], wo_r[:, :, c0:c0 + cn])
                        for ti, (t0, pt) in enumerate(TT):
                            ps = psA.tile([P, 512], F32, tag="pA")
                            for kk in range(KK):
                                nc.tensor.matmul(
                                    ps[:pt], ctx_fm[:, kk, t0:t0 + pt],
                                    wb[:, kk, :], start=(kk == 0), stop=False)
                            nc.tensor.matmul(
                                ps[:pt], ones_r[:, :pt], t_bo[:, c0:c0 + cn],
                                start=False, stop=True)
                            nc.vector.scalar_tensor_tensor(
                                x2[:pt, ti, c0:c0 + cn], ps[:pt], 0.0,
                                xb[:pt, ti, c0:c0 + cn], OP.add, OP.add)
                            if ci == len(DC) - 1:
                                # x2 tile complete: fold its LN2 stats in now
                                ln_tile_stats(lnpool, stats2, x2, ti, pt)



                # ---- stage F: LN2 apply ----
                ln_finalize(stats2, 0, 4)
                ln_apply_tiles(lnpool, stats2, x2, g2_sb, gb2_sb, xn2_fm, (0, 1, 2, 3))
                ln_finalize(stats2, 4, 5)
                ln_apply_tiles(lnpool, stats2, x2, g2_sb, gb2_sb, xn2_fm, (4,))

                # ---- stage G: MLP ----
                with tc.tile_pool(name="mlp", bufs=1) as mpool, \
                     tc.tile_pool(name="wmlp", bufs=2) as mwpool:
                    h1 = mpool.tile([P, FK, SP], F32R, tag="h1")
                    for blk in range(8):
                        wb = mwpool.tile([P, KK, 512], F32R, tag="wmlp")
                        nc.sync.dma_start(wb[:], w1_r[:, :, blk * 512:(blk + 1) * 512])
                        for mi in range(4):
                            m = blk * 4 + mi
                            for (q0, qn) in QC:
                                ps = psA.tile([P, 512], F32, tag="pA")
                                for kk in range(KK):
                                    nc.tensor.matmul(
                                        ps[:, :qn],
                                        wb[:, kk, mi * P:(mi + 1) * P],
                                        xn2_fm[:, kk, q0:q0 + qn],
                                        start=(kk == 0), stop=(kk == KK - 1))
                                nc.scalar.activation(
                                    h1[:, m, q0:q0 + qn], ps[:, :qn],
                                    _GELU, bias=b1_sb[:, m:m + 1])
                    mlp_fm = mpool.tile([P, KK, SP], F32R, tag="mlp_fm")
                    for m in range(KK):
                        wb = mwpool.tile([P, FK, P], F32R, tag="wmlp")
                        nc.sync.dma_start(wb[:], w2_r[:, :, m * P:(m + 1) * P])
                        for (q0, qn) in QC:
                            ps = psA.tile([P, 512], F32, tag="pA")
                            for kk2 in range(FK):
                                nc.tensor.matmul(
                                    ps[:, :qn], wb[:, kk2],
                                    h1[:, kk2, q0:q0 + qn],
                                    start=(kk2 == 0), stop=(kk2 == FK - 1))
                            nc.vector.tensor_scalar_add(
                                mlp_fm[:, m, q0:q0 + qn], ps[:, :qn],
                                b2_sb[:, m:m + 1])
                        # this m's feature rows are complete: transpose back to
                        # token-major, add residual, store (interleaves with the
                        # next m's w2 matmuls)
                        for ti, (t0, pt) in enumerate(TT):
                            rp = min(pt, S - t0)   # skip the pad token on store
                            ps = psA.tile([P, 512], F32R, tag="pA")
                            nc.tensor.transpose(
                                ps[:pt, :P], mlp_fm[:, m, t0:t0 + pt], ident_r[:])
                            og = opool.tile([P, P], F32, tag="ostg", bufs=6)
                            nc.vector.scalar_tensor_tensor(
                                og[:pt], ps[:pt, :P], 0.0,
                                x2[:pt, ti, m * P:(m + 1) * P], OP.add, OP.add)
                            nc.sync.dma_start(
                                y_d[b, t0:t0 + rp, m * P:(m + 1) * P], og[:rp])

    nc.compile()
    return nc


def _get_nc():
    global _NC_CACHE
    if _NC_CACHE is None:
        _NC_CACHE = _build()
    return _NC_CACHE


def kernel(**inputs):
    nc = _get_nc()
    x = np.ascontiguousarray(np.asarray(inputs["x"], dtype=np.float32))
    shared = {
        n: np.ascontiguousarray(np.asarray(inputs[n], dtype=np.float32))
        for n in WEIGHT_NAMES
    }
    in_maps = []
    for i in range(NCORES):
        m = dict(shared)
        m["x"] = np.ascontiguousarray(x[i * BL:(i + 1) * BL])
        in_maps.append(m)
    res = bass_utils.run_bass_kernel_spmd(nc, in_maps, core_ids=list(range(NCORES)))
    y = np.concatenate([res.results[i]["y"] for i in range(NCORES)], axis=0)
    return y.astype(np.float32)


# revision 57
# speedup vs baseline: 10774.9739x; 1.0033x over previous
"""Trainium2 Bass kernel for a dense transformer block (pre-LN attention + GELU MLP).

Strategy: data-parallel over batch across 8 NeuronCores (2 batches/core, no
collectives).  Per core: token-major residual stream with feature-major
activations for matmuls (PE-transpose at the two LayerNorms), fp32r matmuls
(full PE rate), softmax without max-subtraction (scores are O(1) bounded by
construction), PV matmul with a ones-column on V to produce row-sums for free.
"""

import numpy as np

import concourse.bass as bass
import concourse.mybir as mybir
import concourse.tile as tile
from concourse import bacc, bass_utils
from concourse.masks import make_identity

# Problem shape (hardcoded per spec nn_Block_58652073394865)
B, S, D, H, F = 16, 577, 1024, 16, 4096
DH = D // H
NCORES = 8
BL = B // NCORES        # batches per core
P = 128
KK = D // P             # 8 chunks of the model dim
FK = F // P             # 32 chunks of the mlp dim
EPS = 1e-6

# fp32r matmuls require even free-dim counts, so pad tokens 577 -> 578 (one
# zeroed pad token) and use even, overlapping moving-token chunks.
SP = 578
TT = [(0, 128), (128, 128), (256, 128), (384, 128), (512, 66)]   # token tiles (incl pad)
QC = [(0, 290), (288, 290)]                                      # moving-token chunks (even, >=256)
DC = [(0, 512), (512, 512)]                                      # model-dim 512 chunks
VS = 66                                                          # per-head stride in v (64 v + 1 ones + 1 pad)

F32 = mybir.dt.float32
F32R = mybir.dt.float32r
AF = mybir.ActivationFunctionType
OP = mybir.AluOpType

WEIGHT_NAMES = [
    "ln1_g", "ln1_b", "wq", "bq", "wk", "bk", "wv", "bv", "wo", "bo",
    "ln2_g", "ln2_b", "w1", "b1", "w2", "b2",
]

_NC_CACHE = None
# CoreSim doesn't implement the Gelu LUT; tests may swap this for AF.Tanh
_GELU = AF.Gelu


def _build():
    nc = bacc.Bacc("TRN2", target_bir_lowering=False, debug=False,
                   num_devices=NCORES)

    x_d = nc.dram_tensor("x", [BL, S, D], F32, kind="ExternalInput").ap()
    y_d = nc.dram_tensor("y", [BL, S, D], F32, kind="ExternalOutput").ap()
    # weights consumed by matmuls -> declare fp32r (same bits as fp32)
    wq_d = nc.dram_tensor("wq", [D, D], F32R, kind="ExternalInput").ap()
    wk_d = nc.dram_tensor("wk", [D, D], F32R, kind="ExternalInput").ap()
    wv_d = nc.dram_tensor("wv", [D, D], F32R, kind="ExternalInput").ap()
    wo_d = nc.dram_tensor("wo", [D, D], F32R, kind="ExternalInput").ap()
    w1_d = nc.dram_tensor("w1", [D, F], F32R, kind="ExternalInput").ap()
    w2_d = nc.dram_tensor("w2", [F, D], F32R, kind="ExternalInput").ap()
    bv_d = nc.dram_tensor("bv", [D], F32R, kind="ExternalInput").ap()   # folded via K=1 matmul
    bo_d = nc.dram_tensor("bo", [D], F32R, kind="ExternalInput").ap()   # folded via K=1 matmul
    bq_d = nc.dram_tensor("bq", [D], F32, kind="ExternalInput").ap()
    bk_d = nc.dram_tensor("bk", [D], F32, kind="ExternalInput").ap()
    b1_d = nc.dram_tensor("b1", [F], F32, kind="ExternalInput").ap()
    b2_d = nc.dram_tensor("b2", [D], F32, kind="ExternalInput").ap()
    g1_d = nc.dram_tensor("ln1_g", [D], F32, kind="ExternalInput").ap()
    gb1_d = nc.dram_tensor("ln1_b", [D], F32, kind="ExternalInput").ap()
    g2_d = nc.dram_tensor("ln2_g", [D], F32, kind="ExternalInput").ap()
    gb2_d = nc.dram_tensor("ln2_b", [D], F32, kind="ExternalInput").ap()

    wq_r = wq_d.rearrange("(ko p) d -> p ko d", p=P)
    wk_r = wk_d.rearrange("(ko p) d -> p ko d", p=P)
    wv_r = wv_d.rearrange("(ko p) d -> p ko d", p=P)
    wo_r = wo_d.rearrange("(ko p) d -> p ko d", p=P)
    w1_r = w1_d.rearrange("(ko p) d -> p ko d", p=P)
    w2_r = w2_d.rearrange("(ko p) d -> p ko d", p=P)

    with tile.TileContext(nc) as tc:
        with tc.tile_pool(name="const", bufs=1) as cpool, \
             tc.tile_pool(name="resid", bufs=2) as rpool, \
             tc.tile_pool(name="fmbuf", bufs=1) as fmpool, \
             tc.tile_pool(name="ostg", bufs=4) as opool, \
             tc.tile_pool(name="lnp", bufs=2) as lnpool, \
             tc.tile_pool(name="psA", bufs=4, space="PSUM") as psA:

            # ---- constants / small params ----
            # tiles pad to 4KB/partition: pack the small params into few tiles
            cA = cpool.tile([P, 7 * KK + FK], F32, tag="cA")
            bq_sb = cA[:, 0:KK]
            bk_sb = cA[:, KK:2 * KK]
            b2_sb = cA[:, 2 * KK:3 * KK]
            g1_sb = cA[:, 3 * KK:4 * KK]
            gb1_sb = cA[:, 4 * KK:5 * KK]
            g2_sb = cA[:, 5 * KK:6 * KK]
            gb2_sb = cA[:, 6 * KK:7 * KK]
            b1_sb = cA[:, 7 * KK:7 * KK + FK]
            nc.sync.dma_start(bq_sb, bq_d.rearrange("(m p) -> p m", p=P))
            nc.sync.dma_start(bk_sb, bk_d.rearrange("(m p) -> p m", p=P))
            nc.sync.dma_start(b2_sb, b2_d.rearrange("(m p) -> p m", p=P))
            nc.sync.dma_start(g1_sb, g1_d.rearrange("(c p) -> p c", p=P))
            nc.sync.dma_start(gb1_sb, gb1_d.rearrange("(c p) -> p c", p=P))
            nc.sync.dma_start(g2_sb, g2_d.rearrange("(c p) -> p c", p=P))
            nc.sync.dma_start(gb2_sb, gb2_d.rearrange("(c p) -> p c", p=P))
            nc.sync.dma_start(b1_sb, b1_d.rearrange("(m p) -> p m", p=P))

            cB = cpool.tile([P, P + 2], F32, tag="cB")
            ident = cB[:, 0:P]
            epsap = cB[:, P:P + 1]
            onec_f = cB[:, P + 1:P + 2]
            make_identity(nc, ident)
            nc.vector.memset(epsap, EPS)
            nc.vector.memset(onec_f, 1.0)

            ident_r = cpool.tile([P, P], F32R, tag="ident_r")
            nc.vector.tensor_copy(ident_r[:], ident)

            ones_f = cpool.tile([1, P], F32, tag="ones_f")
            nc.vector.memset(ones_f[:], 1.0)
            cD = cpool.tile([1, P + 2 * D], F32R, tag="cD")
            ones_r = cD[:, 0:P]
            t_bo = cD[:, P:P + D]
            t_bv = cD[:, P + D:P + 2 * D]
            nc.vector.tensor_copy(ones_r, ones_f[:])
            nc.sync.dma_start(t_bo, bo_d[None, :])
            nc.sync.dma_start(t_bv, bv_d[None, :])

            # token-major layernorm -> feature-major normalized output
            def ln_new_stats(ln_pool):
                stats = ln_pool.tile([P, 20], F32, tag="stats")
                # last token tile covers only 66 partitions; keep the rest defined
                nc.vector.memset(stats[:, 0:5], 0.0)
                nc.vector.memset(stats[:, 5:10], 1.0)
                return stats

            def ln_tile_stats(ln_pool, stats, src, ti, pt):
                negmu = stats[:, 0:5]
                varD = stats[:, 5:10]
                nc.vector.tensor_reduce(
                    negmu[:pt, ti:ti + 1], src[:pt, ti],
                    mybir.AxisListType.X, OP.add)
                nc.vector.tensor_scalar_mul(
                    negmu[:pt, ti:ti + 1], negmu[:pt, ti:ti + 1], -1.0 / D)
                scr = ln_pool.tile([P, D], F32R, tag="xn_tm", bufs=3)
                nc.scalar.activation(
                    scr[:pt], src[:pt, ti], AF.Square,
                    bias=negmu[:pt, ti:ti + 1], accum_out=varD[:pt, ti:ti + 1])

            def ln_finalize(stats, lo, hi):
                # rsig for tile range [lo, hi)
                nc.scalar.activation(stats[:, 10 + lo:10 + hi],
                                     stats[:, 5 + lo:5 + hi], AF.Sqrt,
                                     scale=1.0 / D, bias=epsap[:])
                nc.vector.reciprocal(stats[:, 15 + lo:15 + hi],
                                     stats[:, 10 + lo:10 + hi])

            def ln_apply_tiles(ln_pool, stats, src, g_sb, gb_sb, dst_fm, tis):
                negmu = stats[:, 0:5]
                rsig = stats[:, 15:20]
                for ti in tis:
                    t0, pt = TT[ti]
                    xn = ln_pool.tile([P, D], F32R, tag="xn_tm", bufs=3)
                    nc.vector.tensor_scalar(
                        xn[:pt], src[:pt, ti],
                        negmu[:pt, ti:ti + 1], rsig[:pt, ti:ti + 1],
                        OP.add, OP.mult)
                    for kk in range(KK):
                        pst = psA.tile([P, 512], F32R, tag="pA")
                        nc.tensor.transpose(
                            pst[:, :pt], xn[:pt, kk * P:(kk + 1) * P],
                            ident_r[:pt, :pt])
                        nc.vector.scalar_tensor_tensor(
                            dst_fm[:, kk, t0:t0 + pt], pst[:, :pt],
                            g_sb[:, kk:kk + 1],
                            gb_sb[:, kk:kk + 1].to_broadcast((P, pt)),
                            OP.mult, OP.add)

            def layer_norm_fm(ln_pool, src, g_sb, gb_sb, dst_fm):
                stats = ln_new_stats(ln_pool)
                for ti, (t0, pt) in enumerate(TT):
                    ln_tile_stats(ln_pool, stats, src, ti, pt)
                ln_finalize(stats, 0, 4)
                ln_apply_tiles(ln_pool, stats, src, g_sb, gb_sb, dst_fm, (0, 1, 2, 3))
                ln_finalize(stats, 4, 5)
                ln_apply_tiles(ln_pool, stats, src, g_sb, gb_sb, dst_fm, (4,))

            for b in range(BL):
                xn_fm = fmpool.tile([P, KK, SP], F32R, tag="xn_fm")
                xb = rpool.tile([P, 5, D], F32, tag="resid")

                # ---- stage A: load x (token-major); zero the pad token row ----
                # (engine start-partition must be a multiple of 32: zero 64..127
                # first, then the DMA rewrites the real rows 0..64)
                nc.vector.memset(xb[64:, 4, :], 0.0)
                for ti, (t0, pt) in enumerate(TT):
                    rp = min(pt, S - t0)   # real (non-pad) tokens in this tile
                    nc.sync.dma_start(xb[:rp, ti], x_d[b, t0:t0 + rp, :])

                # ---- stage B: LN1 -> xn_fm ----
                layer_norm_fm(lnpool, xb, g1_sb, gb1_sb, xn_fm)

                with tc.tile_pool(name="attn", bufs=1) as apool, \
                     tc.tile_pool(name="wblk", bufs=2) as wpool:
                    q_fm = apool.tile([P, KK, SP], F32R, tag="q")
                    k_fm = apool.tile([P, KK, SP], F32R, tag="k")
                    v_sb = apool.tile([P, 5, H * VS], F32R, tag="v")
                    ctx_fm = apool.tile([P, KK, SP], F32R, tag="ctx")

                    # col 64 of each head's stride-66 group = 1 (rowsum trick),
                    # col 65 = 0 (fp32r even-M pad).  The pad token's whole v
                    # row (tile 4, partition 65) must be zero: zero partitions
                    # 64.. first, later writes refill only the real rows.
                    v_hc = v_sb[:].rearrange("p t (h c) -> p t h c", c=VS)
                    # memset can't target fp32r; zero via a uint32 view
                    nc.vector.memset(v_hc[64:, 4:5].bitcast(mybir.dt.uint32), 0)
                    nc.vector.memset(v_hc[:, :, :, 65:66].bitcast(mybir.dt.uint32), 0)
                    nc.vector.tensor_copy(
                        v_hc[:, 0:4, :, 64:65],
                        onec_f[:, :, None, None].to_broadcast((P, 4, H, 1)))
                    nc.vector.tensor_copy(
                        v_hc[:65, 4:5, :, 64:65],
                        onec_f[:65, :, None, None].to_broadcast((65, 1, H, 1)))

                    # ---- stage C/D interleaved: projections + attention ----
                    # blk covers q/k m-tiles 4*blk..4*blk+3 and v heads
                    # 8*blk..8*blk+7 == attention heads 8*blk..8*blk+7, so each
                    # half's projections feed its attention while the NEXT
                    # half's projection matmuls fill the exp-bound PE idle.
                    def emit_qk(blk):
                        for w_r, bias_sb, dst in ((wq_r, bq_sb, q_fm), (wk_r, bk_sb, k_fm)):
                            wb = wpool.tile([P, KK, 512], F32R, tag="wblk")
                            nc.sync.dma_start(wb[:], w_r[:, :, blk * 512:(blk + 1) * 512])
                            for mi in range(4):
                                m = blk * 4 + mi
                                for (q0, qn) in QC:
                                    ps = psA.tile([P, 512], F32, tag="pA")
                                    for kk in range(KK):
                                        nc.tensor.matmul(
                                            ps[:, :qn],
                                            wb[:, kk, mi * P:(mi + 1) * P],
                                            xn_fm[:, kk, q0:q0 + qn],
                                            start=(kk == 0), stop=(kk == KK - 1))
                                    nc.scalar.activation(
                                        dst[:, m, q0:q0 + qn], ps[:, :qn],
                                        AF.Identity, bias=bias_sb[:, m:m + 1])

                    def emit_v(ci):
                        c0, cn = DC[ci]
                        wb = wpool.tile([P, KK, 512], F32R, tag="wblk")
                        nc.sync.dma_start(wb[:], wv_r[:, :, c0:c0 + cn])
                        for ti, (t0, pt) in enumerate(TT):
                            ps = psA.tile([P, 512], F32, tag="pA")
                            for kk in range(KK):
                                nc.tensor.matmul(
                                    ps[:pt], xn_fm[:, kk, t0:t0 + pt],
                                    wb[:, kk, :], start=(kk == 0), stop=False)
                            nc.tensor.matmul(
                                ps[:pt], ones_r[:, :pt], t_bv[:, c0:c0 + cn],
                                start=False, stop=True)
                            rp = min(pt, S - t0)
                            nc.vector.tensor_copy(
                                v_sb[:rp, ti].rearrange("p (h c) -> p h c", c=VS)[:, ci * 8:(ci + 1) * 8, 0:64],
                                ps[:rp, :cn].rearrange("p (h c) -> p h c", c=64))

                    def emit_attn(h):
                        hrow = (h % 2) * 64
                        kkh = h // 2
                        for qi, (q0, qn) in enumerate(QC):
                            es = apool.tile([P, 5, qn], F32R, tag=f"es{qi}")
                            # pair the 5 score tiles into 2-bank psum groups so
                            # each Exp covers 2 tiles (halves the per-op cost)
                            for pair in ((0, 1), (2, 3), (4,)):
                                pg = psA.tile([P, 2, 512], F32, tag="pS", bufs=2)
                                for j, kt in enumerate(pair):
                                    t0, ptk = TT[kt]
                                    nc.tensor.matmul(
                                        pg[:ptk, j, :qn],
                                        k_fm[hrow:hrow + 64, kkh, t0:t0 + ptk],
                                        q_fm[hrow:hrow + 64, kkh, q0:q0 + qn],
                                        start=True, stop=True)
                                npair = len(pair)
                                prow = TT[pair[0]][1]   # 128 for full pairs, 66 for (4,)
                                nc.scalar.activation(
                                    es[:prow, pair[0]:pair[0] + npair, :],
                                    pg[:prow, :npair, :qn],
                                    AF.Exp, scale=1.0 / np.sqrt(DH))
                            pc = psA.tile([VS, 512], F32, tag="pA")
                            for kt, (t0, ptk) in enumerate(TT):
                                nc.tensor.matmul(
                                    pc[:, :qn],
                                    v_sb[:ptk, kt, h * VS:(h + 1) * VS],
                                    es[:ptk, kt, :],
                                    start=(kt == 0), stop=(kt == 4))
                            rc = apool.tile([1, 290], F32, tag="rc", bufs=2)
                            nc.vector.reciprocal(rc[:, :qn], pc[64:65, :qn])
                            rb = apool.tile([64, 290], F32, tag="rb", bufs=2)
                            nc.gpsimd.partition_broadcast(rb[:, :qn], rc[:, :qn])
                            nc.vector.tensor_tensor(
                                ctx_fm[hrow:hrow + 64, kkh, q0:q0 + qn],
                                pc[0:64, :qn], rb[:, :qn], OP.mult)

                    emit_qk(0)
                    emit_v(0)
                    for h in range(8):
                        emit_attn(h)
                    emit_qk(1)
                    emit_v(1)
                    for h in range(8, H):
                        emit_attn(h)

                    # ---- stage E: output projection + residual -> x2,
                    # with LN2 folded in per-tile ----
                    x2 = rpool.tile([P, 5, D], F32, tag="resid")
                    xn2_fm = fmpool.tile([P, KK, SP], F32R, tag="xn_fm")
                    stats2 = ln_new_stats(lnpool)
                    for ci, (c0, cn) in enumerate(DC):
                        wb = wpool.tile([P, KK, 512], F32R, tag="wblk")
                        nc.sync.dma_start(wb[:], wo_r[:, :, c0:c0 + cn])
                        for ti, (t0, pt) in enumerate(TT):
                            ps = psA.tile([P, 512], F32, tag="pA")
                            for kk in range(KK):
                                nc.tensor.matmul(
                                    ps[:pt], ctx_fm[:, kk, t0:t0 + pt],
                                    wb[:, kk, :], start=(kk == 0), stop=False)
                            nc.tensor.matmul(
                                ps[:pt], ones_r[:, :pt], t_bo[:, c0:c0 + cn],
                                start=False, stop=True)
                            nc.vector.scalar_tensor_tensor(
                                x2[:pt, ti, c0:c0 + cn], ps[:pt], 0.0,
                                xb[:pt, ti, c0:c0 + cn], OP.add, OP.add)
                            if ci == len(DC) - 1:
                                # x2 tile complete: fold its LN2 stats in now
                                ln_tile_stats(lnpool, stats2, x2, ti, pt)



                # ---- stage F: LN2 apply ----
                ln_finalize(stats2, 0, 4)
                ln_apply_tiles(lnpool, stats2, x2, g2_sb, gb2_sb, xn2_fm, (0, 1, 2, 3))
                ln_finalize(stats2, 4, 5)
                ln_apply_tiles(lnpool, stats2, x2, g2_sb, gb2_sb, xn2_fm, (4,))

                # ---- stage G: MLP ----
                with tc.tile_pool(name="mlp", bufs=1) as mpool, \
                     tc.tile_pool(name="wmlp", bufs=2) as mwpool:
                    h1 = mpool.tile([P, FK, SP], F32R, tag="h1")
                    _psc = [0]

                    def mlp_psum():
                        # pS's 2x2 banks are idle during MLP: every 3rd group
                        # borrows one -> 6 accumulation groups in flight
                        _psc[0] += 1
                        if _psc[0] % 3 == 0:
                            t = psA.tile([P, 2, 512], F32, tag="pS", bufs=2,
                                         name="ps_alt")
                            return t[:, 0]
                        return psA.tile([P, 512], F32, tag="pA", name="ps_a")

                    for blk in range(8):
                        wb = mwpool.tile([P, KK, 512], F32R, tag="wmlp")
                        nc.sync.dma_start(wb[:], w1_r[:, :, blk * 512:(blk + 1) * 512])
                        for mi in range(4):
                            m = blk * 4 + mi
                            for (q0, qn) in QC:
                                ps = mlp_psum()
                                for kk in range(KK):
                                    nc.tensor.matmul(
                                        ps[:, :qn],
                                        wb[:, kk, mi * P:(mi + 1) * P],
                                        xn2_fm[:, kk, q0:q0 + qn],
                                        start=(kk == 0), stop=(kk == KK - 1))
                                nc.scalar.activation(
                                    h1[:, m, q0:q0 + qn], ps[:, :qn],
                                    _GELU, bias=b1_sb[:, m:m + 1])
                    mlp_fm = mpool.tile([P, KK, SP], F32R, tag="mlp_fm")
                    for m in range(KK):
                        wb = mwpool.tile([P, FK, P], F32R, tag="wmlp")
                        nc.sync.dma_start(wb[:], w2_r[:, :, m * P:(m + 1) * P])
                        for (q0, qn) in QC:
                            ps = mlp_psum()
                            for kk2 in range(FK):
                                nc.tensor.matmul(
                                    ps[:, :qn], wb[:, kk2],
                                    h1[:, kk2, q0:q0 + qn],
                                    start=(kk2 == 0), stop=(kk2 == FK - 1))
                            nc.vector.tensor_scalar_add(
                                mlp_fm[:, m, q0:q0 + qn], ps[:, :qn],
                                b2_sb[:, m:m + 1])
                        # this m's feature rows are complete: transpose back to
                        # token-major, add residual, store (interleaves with the
                        # next m's w2 matmuls)
                        for ti, (t0, pt) in enumerate(TT):
                            rp = min(pt, S - t0)   # skip the pad token on store
                            ps = psA.tile([P, 512], F32R, tag="pA")
                            nc.tensor.transpose(
                                ps[:pt, :P], mlp_fm[:, m, t0:t0 + pt], ident_r[:])
                            og = opool.tile([P, P], F32, tag="ostg", bufs=6)
                            nc.vector.scalar_tensor_tensor(
                                og[:pt], ps[:pt, :P], 0.0,
                                x2[:pt, ti, m * P:(m + 1) * P], OP.add, OP.add)
                            nc.sync.dma_start(
                                y_d[b, t0:t0 + rp, m * P:(m + 1) * P], og[:rp])

    nc.compile()
    return nc


def _get_nc():
    global _NC_CACHE
    if _NC_CACHE is None:
        _NC_CACHE = _build()
    return _NC_CACHE


def kernel(**inputs):
    nc = _get_nc()
    x = np.ascontiguousarray(np.asarray(inputs["x"], dtype=np.float32))
    shared = {
        n: np.ascontiguousarray(np.asarray(inputs[n], dtype=np.float32))
        for n in WEIGHT_NAMES
    }
    in_maps = []
    for i in range(NCORES):
        m = dict(shared)
        m["x"] = np.ascontiguousarray(x[i * BL:(i + 1) * BL])
        in_maps.append(m)
    res = bass_utils.run_bass_kernel_spmd(nc, in_maps, core_ids=list(range(NCORES)))
    y = np.concatenate([res.results[i]["y"] for i in range(NCORES)], axis=0)
    return y.astype(np.float32)


# revision 60
# speedup vs baseline: 10799.0910x; 1.0022x over previous
"""Trainium2 Bass kernel for a dense transformer block (pre-LN attention + GELU MLP).

Strategy: data-parallel over batch across 8 NeuronCores (2 batches/core, no
collectives).  Per core: token-major residual stream with feature-major
activations for matmuls (PE-transpose at the two LayerNorms), fp32r matmuls
(full PE rate), softmax without max-subtraction (scores are O(1) bounded by
construction), PV matmul with a ones-column on V to produce row-sums for free.
"""

import numpy as np

import concourse.bass as bass
import concourse.mybir as mybir
import concourse.tile as tile
from concourse import bacc, bass_utils
from concourse.masks import make_identity

# Problem shape (hardcoded per spec nn_Block_58652073394865)
B, S, D, H, F = 16, 577, 1024, 16, 4096
DH = D // H
NCORES = 8
BL = B // NCORES        # batches per core
P = 128
KK = D // P             # 8 chunks of the model dim
FK = F // P             # 32 chunks of the mlp dim
EPS = 1e-6

# fp32r matmuls require even free-dim counts, so pad tokens 577 -> 578 (one
# zeroed pad token) and use even, overlapping moving-token chunks.
SP = 578
TT = [(0, 128), (128, 128), (256, 128), (384, 128), (512, 66)]   # token tiles (incl pad)
QC = [(0, 290), (288, 290)]                                      # moving-token chunks (even, >=256)
DC = [(0, 512), (512, 512)]                                      # model-dim 512 chunks
VS = 66                                                          # per-head stride in v (64 v + 1 ones + 1 pad)

F32 = mybir.dt.float32
F32R = mybir.dt.float32r
AF = mybir.ActivationFunctionType
OP = mybir.AluOpType

WEIGHT_NAMES = [
    "ln1_g", "ln1_b", "wq", "bq", "wk", "bk", "wv", "bv", "wo", "bo",
    "ln2_g", "ln2_b", "w1", "b1", "w2", "b2",
]

_NC_CACHE = None
# CoreSim doesn't implement the Gelu LUT; tests may swap this for AF.Tanh
_GELU = AF.Gelu


def _build():
    nc = bacc.Bacc("TRN2", target_bir_lowering=False, debug=False,
                   num_devices=NCORES)

    x_d = nc.dram_tensor("x", [BL, S, D], F32, kind="ExternalInput").ap()
    y_d = nc.dram_tensor("y", [BL, S, D], F32, kind="ExternalOutput").ap()
    # weights consumed by matmuls -> declare fp32r (same bits as fp32)
    wq_d = nc.dram_tensor("wq", [D, D], F32R, kind="ExternalInput").ap()
    wk_d = nc.dram_tensor("wk", [D, D], F32R, kind="ExternalInput").ap()
    wv_d = nc.dram_tensor("wv", [D, D], F32R, kind="ExternalInput").ap()
    wo_d = nc.dram_tensor("wo", [D, D], F32R, kind="ExternalInput").ap()
    w1_d = nc.dram_tensor("w1", [D, F], F32R, kind="ExternalInput").ap()
    w2_d = nc.dram_tensor("w2", [F, D], F32R, kind="ExternalInput").ap()
    bv_d = nc.dram_tensor("bv", [D], F32R, kind="ExternalInput").ap()   # folded via K=1 matmul
    bo_d = nc.dram_tensor("bo", [D], F32R, kind="ExternalInput").ap()   # folded via K=1 matmul
    bq_d = nc.dram_tensor("bq", [D], F32, kind="ExternalInput").ap()
    bk_d = nc.dram_tensor("bk", [D], F32, kind="ExternalInput").ap()
    b1_d = nc.dram_tensor("b1", [F], F32, kind="ExternalInput").ap()
    b2_d = nc.dram_tensor("b2", [D], F32, kind="ExternalInput").ap()
    g1_d = nc.dram_tensor("ln1_g", [D], F32, kind="ExternalInput").ap()
    gb1_d = nc.dram_tensor("ln1_b", [D], F32, kind="ExternalInput").ap()
    g2_d = nc.dram_tensor("ln2_g", [D], F32, kind="ExternalInput").ap()
    gb2_d = nc.dram_tensor("ln2_b", [D], F32, kind="ExternalInput").ap()

    wq_r = wq_d.rearrange("(ko p) d -> p ko d", p=P)
    wk_r = wk_d.rearrange("(ko p) d -> p ko d", p=P)
    wv_r = wv_d.rearrange("(ko p) d -> p ko d", p=P)
    wo_r = wo_d.rearrange("(ko p) d -> p ko d", p=P)
    w1_r = w1_d.rearrange("(ko p) d -> p ko d", p=P)
    w2_r = w2_d.rearrange("(ko p) d -> p ko d", p=P)

    with tile.TileContext(nc) as tc:
        with tc.tile_pool(name="const", bufs=1) as cpool, \
             tc.tile_pool(name="resid", bufs=2) as rpool, \
             tc.tile_pool(name="fmbuf", bufs=1) as fmpool, \
             tc.tile_pool(name="ostg", bufs=4) as opool, \
             tc.tile_pool(name="lnp", bufs=2) as lnpool, \
             tc.tile_pool(name="psA", bufs=4, space="PSUM") as psA:

            # ---- constants / small params ----
            # tiles pad to 4KB/partition: pack the small params into few tiles
            cA = cpool.tile([P, 7 * KK + FK], F32, tag="cA")
            bq_sb = cA[:, 0:KK]
            bk_sb = cA[:, KK:2 * KK]
            b2_sb = cA[:, 2 * KK:3 * KK]
            g1_sb = cA[:, 3 * KK:4 * KK]
            gb1_sb = cA[:, 4 * KK:5 * KK]
            g2_sb = cA[:, 5 * KK:6 * KK]
            gb2_sb = cA[:, 6 * KK:7 * KK]
            b1_sb = cA[:, 7 * KK:7 * KK + FK]
            nc.sync.dma_start(bq_sb, bq_d.rearrange("(m p) -> p m", p=P))
            nc.sync.dma_start(bk_sb, bk_d.rearrange("(m p) -> p m", p=P))
            nc.sync.dma_start(b2_sb, b2_d.rearrange("(m p) -> p m", p=P))
            nc.sync.dma_start(g1_sb, g1_d.rearrange("(c p) -> p c", p=P))
            nc.sync.dma_start(gb1_sb, gb1_d.rearrange("(c p) -> p c", p=P))
            nc.sync.dma_start(g2_sb, g2_d.rearrange("(c p) -> p c", p=P))
            nc.sync.dma_start(gb2_sb, gb2_d.rearrange("(c p) -> p c", p=P))
            nc.sync.dma_start(b1_sb, b1_d.rearrange("(m p) -> p m", p=P))

            cB = cpool.tile([P, P + 2], F32, tag="cB")
            ident = cB[:, 0:P]
            epsap = cB[:, P:P + 1]
            onec_f = cB[:, P + 1:P + 2]
            make_identity(nc, ident)
            nc.vector.memset(epsap, EPS)
            nc.vector.memset(onec_f, 1.0)

            ident_r = cpool.tile([P, P], F32R, tag="ident_r")
            nc.vector.tensor_copy(ident_r[:], ident)

            ones_f = cpool.tile([1, P], F32, tag="ones_f")
            nc.vector.memset(ones_f[:], 1.0)
            cD = cpool.tile([1, P + 2 * D], F32R, tag="cD")
            ones_r = cD[:, 0:P]
            t_bo = cD[:, P:P + D]
            t_bv = cD[:, P + D:P + 2 * D]
            nc.vector.tensor_copy(ones_r, ones_f[:])
            nc.sync.dma_start(t_bo, bo_d[None, :])
            nc.sync.dma_start(t_bv, bv_d[None, :])

            # token-major layernorm -> feature-major normalized output
            def ln_new_stats(ln_pool):
                stats = ln_pool.tile([P, 20], F32, tag="stats")
                # last token tile covers only 66 partitions; keep the rest defined
                nc.vector.memset(stats[:, 0:5], 0.0)
                nc.vector.memset(stats[:, 5:10], 1.0)
                return stats

            def ln_tile_stats(ln_pool, stats, src, ti, pt):
                negmu = stats[:, 0:5]
                varD = stats[:, 5:10]
                nc.vector.tensor_reduce(
                    negmu[:pt, ti:ti + 1], src[:pt, ti],
                    mybir.AxisListType.X, OP.add)
                nc.vector.tensor_scalar_mul(
                    negmu[:pt, ti:ti + 1], negmu[:pt, ti:ti + 1], -1.0 / D)
                scr = ln_pool.tile([P, D], F32R, tag="xn_tm", bufs=3)
                nc.scalar.activation(
                    scr[:pt], src[:pt, ti], AF.Square,
                    bias=negmu[:pt, ti:ti + 1], accum_out=varD[:pt, ti:ti + 1])

            def ln_finalize(stats, lo, hi):
                # rsig for tile range [lo, hi)
                nc.scalar.activation(stats[:, 10 + lo:10 + hi],
                                     stats[:, 5 + lo:5 + hi], AF.Sqrt,
                                     scale=1.0 / D, bias=epsap[:])
                nc.vector.reciprocal(stats[:, 15 + lo:15 + hi],
                                     stats[:, 10 + lo:10 + hi])

            def ln_apply_tiles(ln_pool, stats, src, g_sb, gb_sb, dst_fm, tis):
                negmu = stats[:, 0:5]
                rsig = stats[:, 15:20]
                for ti in tis:
                    t0, pt = TT[ti]
                    xn = ln_pool.tile([P, D], F32R, tag="xn_tm", bufs=3)
                    nc.vector.tensor_scalar(
                        xn[:pt], src[:pt, ti],
                        negmu[:pt, ti:ti + 1], rsig[:pt, ti:ti + 1],
                        OP.add, OP.mult)
                    for kk in range(KK):
                        pst = psA.tile([P, 512], F32R, tag="pA")
                        nc.tensor.transpose(
                            pst[:, :pt], xn[:pt, kk * P:(kk + 1) * P],
                            ident_r[:pt, :pt])
                        nc.vector.scalar_tensor_tensor(
                            dst_fm[:, kk, t0:t0 + pt], pst[:, :pt],
                            g_sb[:, kk:kk + 1],
                            gb_sb[:, kk:kk + 1].to_broadcast((P, pt)),
                            OP.mult, OP.add)

            def layer_norm_fm(ln_pool, src, g_sb, gb_sb, dst_fm):
                stats = ln_new_stats(ln_pool)
                for ti, (t0, pt) in enumerate(TT):
                    ln_tile_stats(ln_pool, stats, src, ti, pt)
                # finalize tile 0 alone so its transposes start after one x-tile
                ln_finalize(stats, 0, 1)
                ln_apply_tiles(ln_pool, stats, src, g_sb, gb_sb, dst_fm, (0,))
                ln_finalize(stats, 1, 4)
                ln_apply_tiles(ln_pool, stats, src, g_sb, gb_sb, dst_fm, (1, 2, 3))
                ln_finalize(stats, 4, 5)
                ln_apply_tiles(ln_pool, stats, src, g_sb, gb_sb, dst_fm, (4,))

            for b in range(BL):
                xn_fm = fmpool.tile([P, KK, SP], F32R, tag="xn_fm")
                xb = rpool.tile([P, 5, D], F32, tag="resid")

                # ---- stage A: load x (token-major); zero the pad token row ----
                # (engine start-partition must be a multiple of 32: zero 64..127
                # first, then the DMA rewrites the real rows 0..64)
                nc.vector.memset(xb[64:, 4, :], 0.0)
                for ti, (t0, pt) in enumerate(TT):
                    rp = min(pt, S - t0)   # real (non-pad) tokens in this tile
                    nc.sync.dma_start(xb[:rp, ti], x_d[b, t0:t0 + rp, :])

                # ---- stage B: LN1 -> xn_fm ----
                layer_norm_fm(lnpool, xb, g1_sb, gb1_sb, xn_fm)

                with tc.tile_pool(name="attn", bufs=1) as apool, \
                     tc.tile_pool(name="wblk", bufs=2) as wpool:
                    q_fm = apool.tile([P, KK, SP], F32R, tag="q")
                    k_fm = apool.tile([P, KK, SP], F32R, tag="k")
                    v_sb = apool.tile([P, 5, H * VS], F32R, tag="v")
                    ctx_fm = apool.tile([P, KK, SP], F32R, tag="ctx")

                    # col 64 of each head's stride-66 group = 1 (rowsum trick),
                    # col 65 = 0 (fp32r even-M pad).  The pad token's whole v
                    # row (tile 4, partition 65) must be zero: zero partitions
                    # 64.. first, later writes refill only the real rows.
                    v_hc = v_sb[:].rearrange("p t (h c) -> p t h c", c=VS)
                    # memset can't target fp32r; zero via a uint32 view
                    nc.vector.memset(v_hc[64:, 4:5].bitcast(mybir.dt.uint32), 0)
                    nc.vector.memset(v_hc[:, :, :, 65:66].bitcast(mybir.dt.uint32), 0)
                    nc.vector.tensor_copy(
                        v_hc[:, 0:4, :, 64:65],
                        onec_f[:, :, None, None].to_broadcast((P, 4, H, 1)))
                    nc.vector.tensor_copy(
                        v_hc[:65, 4:5, :, 64:65],
                        onec_f[:65, :, None, None].to_broadcast((65, 1, H, 1)))

                    # ---- stage C/D interleaved: projections + attention ----
                    # blk covers q/k m-tiles 4*blk..4*blk+3 and v heads
                    # 8*blk..8*blk+7 == attention heads 8*blk..8*blk+7, so each
                    # half's projections feed its attention while the NEXT
                    # half's projection matmuls fill the exp-bound PE idle.
                    def emit_qk(blk):
                        for w_r, bias_sb, dst in ((wq_r, bq_sb, q_fm), (wk_r, bk_sb, k_fm)):
                            wb = wpool.tile([P, KK, 512], F32R, tag="wblk")
                            nc.sync.dma_start(wb[:], w_r[:, :, blk * 512:(blk + 1) * 512])
                            for mi in range(4):
                                m = blk * 4 + mi
                                for (q0, qn) in QC:
                                    ps = psA.tile([P, 512], F32, tag="pA")
                                    for kk in range(KK):
                                        nc.tensor.matmul(
                                            ps[:, :qn],
                                            wb[:, kk, mi * P:(mi + 1) * P],
                                            xn_fm[:, kk, q0:q0 + qn],
                                            start=(kk == 0), stop=(kk == KK - 1))
                                    nc.scalar.activation(
                                        dst[:, m, q0:q0 + qn], ps[:, :qn],
                                        AF.Identity, bias=bias_sb[:, m:m + 1])

                    def emit_v(ci):
                        c0, cn = DC[ci]
                        wb = wpool.tile([P, KK, 512], F32R, tag="wblk")
                        nc.sync.dma_start(wb[:], wv_r[:, :, c0:c0 + cn])
                        for ti, (t0, pt) in enumerate(TT):
                            ps = psA.tile([P, 512], F32, tag="pA")
                            for kk in range(KK):
                                nc.tensor.matmul(
                                    ps[:pt], xn_fm[:, kk, t0:t0 + pt],
                                    wb[:, kk, :], start=(kk == 0), stop=False)
                            nc.tensor.matmul(
                                ps[:pt], ones_r[:, :pt], t_bv[:, c0:c0 + cn],
                                start=False, stop=True)
                            rp = min(pt, S - t0)
                            nc.vector.tensor_copy(
                                v_sb[:rp, ti].rearrange("p (h c) -> p h c", c=VS)[:, ci * 8:(ci + 1) * 8, 0:64],
                                ps[:rp, :cn].rearrange("p (h c) -> p h c", c=64))

                    def emit_attn(h):
                        hrow = (h % 2) * 64
                        kkh = h // 2
                        for qi, (q0, qn) in enumerate(QC):
                            es = apool.tile([P, 5, qn], F32R, tag=f"es{qi}")
                            # pair the 5 score tiles into 2-bank psum groups so
                            # each Exp covers 2 tiles (halves the per-op cost)
                            for pair in ((0, 1), (2, 3), (4,)):
                                pg = psA.tile([P, 2, 512], F32, tag="pS", bufs=2)
                                for j, kt in enumerate(pair):
                                    t0, ptk = TT[kt]
                                    nc.tensor.matmul(
                                        pg[:ptk, j, :qn],
                                        k_fm[hrow:hrow + 64, kkh, t0:t0 + ptk],
                                        q_fm[hrow:hrow + 64, kkh, q0:q0 + qn],
                                        start=True, stop=True)
                                npair = len(pair)
                                prow = TT[pair[0]][1]   # 128 for full pairs, 66 for (4,)
                                nc.scalar.activation(
                                    es[:prow, pair[0]:pair[0] + npair, :],
                                    pg[:prow, :npair, :qn],
                                    AF.Exp, scale=1.0 / np.sqrt(DH))
                            pc = psA.tile([VS, 512], F32, tag="pA")
                            for kt, (t0, ptk) in enumerate(TT):
                                nc.tensor.matmul(
                                    pc[:, :qn],
                                    v_sb[:ptk, kt, h * VS:(h + 1) * VS],
                                    es[:ptk, kt, :],
                                    start=(kt == 0), stop=(kt == 4))
                            rc = apool.tile([1, 290], F32, tag="rc", bufs=2)
                            nc.vector.reciprocal(rc[:, :qn], pc[64:65, :qn])
                            rb = apool.tile([64, 290], F32, tag="rb", bufs=2)
                            nc.gpsimd.partition_broadcast(rb[:, :qn], rc[:, :qn])
                            nc.vector.tensor_tensor(
                                ctx_fm[hrow:hrow + 64, kkh, q0:q0 + qn],
                                pc[0:64, :qn], rb[:, :qn], OP.mult)

                    emit_qk(0)
                    emit_v(0)
                    for h in range(8):
                        emit_attn(h)
                    emit_qk(1)
                    emit_v(1)
                    for h in range(8, H):
                        emit_attn(h)

                    # ---- stage E: output projection + residual -> x2,
                    # with LN2 folded in per-tile ----
                    x2 = rpool.tile([P, 5, D], F32, tag="resid")
                    xn2_fm = fmpool.tile([P, KK, SP], F32R, tag="xn_fm")
                    stats2 = ln_new_stats(lnpool)
                    for ci, (c0, cn) in enumerate(DC):
                        wb = wpool.tile([P, KK, 512], F32R, tag="wblk")
                        nc.sync.dma_start(wb[:], wo_r[:, :, c0:c0 + cn])
                        for ti, (t0, pt) in enumerate(TT):
                            ps = psA.tile([P, 512], F32, tag="pA")
                            for kk in range(KK):
                                nc.tensor.matmul(
                                    ps[:pt], ctx_fm[:, kk, t0:t0 + pt],
                                    wb[:, kk, :], start=(kk == 0), stop=False)
                            nc.tensor.matmul(
                                ps[:pt], ones_r[:, :pt], t_bo[:, c0:c0 + cn],
                                start=False, stop=True)
                            nc.vector.scalar_tensor_tensor(
                                x2[:pt, ti, c0:c0 + cn], ps[:pt], 0.0,
                                xb[:pt, ti, c0:c0 + cn], OP.add, OP.add)
                            if ci == len(DC) - 1:
                                # x2 tile complete: fold its LN2 stats in now
                                ln_tile_stats(lnpool, stats2, x2, ti, pt)



                # ---- stage F: LN2 apply ----
                ln_finalize(stats2, 0, 4)
                ln_apply_tiles(lnpool, stats2, x2, g2_sb, gb2_sb, xn2_fm, (0, 1, 2, 3))
                ln_finalize(stats2, 4, 5)
                ln_apply_tiles(lnpool, stats2, x2, g2_sb, gb2_sb, xn2_fm, (4,))

                # ---- stage G: MLP ----
                with tc.tile_pool(name="mlp", bufs=1) as mpool, \
                     tc.tile_pool(name="wmlp", bufs=2) as mwpool:
                    h1 = mpool.tile([P, FK, SP], F32R, tag="h1")
                    _psc = [0]

                    def mlp_psum():
                        # pS's 2x2 banks are idle during MLP: every 3rd group
                        # borrows one -> 6 accumulation groups in flight
                        _psc[0] += 1
                        if _psc[0] % 3 == 0:
                            t = psA.tile([P, 2, 512], F32, tag="pS", bufs=2,
                                         name="ps_alt")
                            return t[:, 0]
                        return psA.tile([P, 512], F32, tag="pA", name="ps_a")

                    for blk in range(8):
                        wb = mwpool.tile([P, KK, 512], F32R, tag="wmlp")
                        nc.sync.dma_start(wb[:], w1_r[:, :, blk * 512:(blk + 1) * 512])
                        for mi in range(4):
                            m = blk * 4 + mi
                            for (q0, qn) in QC:
                                ps = mlp_psum()
                                for kk in range(KK):
                                    nc.tensor.matmul(
                                        ps[:, :qn],
                                        wb[:, kk, mi * P:(mi + 1) * P],
                                        xn2_fm[:, kk, q0:q0 + qn],
                                        start=(kk == 0), stop=(kk == KK - 1))
                                nc.scalar.activation(
                                    h1[:, m, q0:q0 + qn], ps[:, :qn],
                                    _GELU, bias=b1_sb[:, m:m + 1])
                    mlp_fm = mpool.tile([P, KK, SP], F32R, tag="mlp_fm")
                    for m in range(KK):
                        wb = mwpool.tile([P, FK, P], F32R, tag="wmlp")
                        nc.sync.dma_start(wb[:], w2_r[:, :, m * P:(m + 1) * P])
                        for (q0, qn) in QC:
                            ps = mlp_psum()
                            for kk2 in range(FK):
                                nc.tensor.matmul(
                                    ps[:, :qn], wb[:, kk2],
                                    h1[:, kk2, q0:q0 + qn],
                                    start=(kk2 == 0), stop=(kk2 == FK - 1))
                            nc.vector.tensor_scalar_add(
                                mlp_fm[:, m, q0:q0 + qn], ps[:, :qn],
                                b2_sb[:, m:m + 1])
                        # this m's feature rows are complete: transpose back to
                        # token-major, add residual, store (interleaves with the
                        # next m's w2 matmuls)
                        for ti, (t0, pt) in enumerate(TT):
                            rp = min(pt, S - t0)   # skip the pad token on store
                            ps = psA.tile([P, 512], F32R, tag="pA")
                            nc.tensor.transpose(
                                ps[:pt, :P], mlp_fm[:, m, t0:t0 + pt], ident_r[:])
                            og = opool.tile([P, P], F32, tag="ostg", bufs=6)
                            nc.vector.scalar_tensor_tensor(
                                og[:pt], ps[:pt, :P], 0.0,
                                x2[:pt, ti, m * P:(m + 1) * P], OP.add, OP.add)
                            nc.sync.dma_start(
                                y_d[b, t0:t0 + rp, m * P:(m + 1) * P], og[:rp])

    nc.compile()
    return nc


def _get_nc():
    global _NC_CACHE
    if _NC_CACHE is None:
        _NC_CACHE = _build()
    return _NC_CACHE


def kernel(**inputs):
    nc = _get_nc()
    x = np.ascontiguousarray(np.asarray(inputs["x"], dtype=np.float32))
    shared = {
        n: np.ascontiguousarray(np.asarray(inputs[n], dtype=np.float32))
        for n in WEIGHT_NAMES
    }
    in_maps = []
    for i in range(NCORES):
        m = dict(shared)
        m["x"] = np.ascontiguousarray(x[i * BL:(i + 1) * BL])
        in_maps.append(m)
    res = bass_utils.run_bass_kernel_spmd(nc, in_maps, core_ids=list(range(NCORES)))
    y = np.concatenate([res.results[i]["y"] for i in range(NCORES)], axis=0)
    return y.astype(np.float32)
